# revision 56
# baseline (speedup 1.0000x reference)
"""Trainium2 Bass kernel for nn_FullAttention_17789754540074.

Self-contained: takes the FULL inputs of reference.setup_inputs(), returns the
FULL output. Internally shards across 8 NeuronCores as 2-way data parallel
(batch) x 4-way tensor parallel (3 heads + 384 FF pairs per rank), runs one
SPMD Bass/Tile program via run_bass_kernel_spmd, and sums the 4 partial
outputs per batch on the host (the unshard step for partial-sum TP sharding).

Design notes (vs the earlier fp32r version; 265.5us -> 185.0us sim time):
  - bf16 activations/weights everywhere off the critical-precision path; x,
    PSUM accumulation, RMS stats, rk/rq rstds and the residual merge stay
    fp32 (hw rel err 1.8e-3).
  - per-channel-chunk RMS stats + interleaved x/wf chunk DMAs; aux tables
    ride the ACT DGE queue so the SP queue streams x/wf back-to-back.
  - single flat SBUF scope with emission order = scheduler priority:
    q01/k01 blocks -> their stats -> half-granular ropes -> q2/k2 -> v ->
    attention (hf-major) -> ffx/gate blocks -> output blocks.  The ffx/gate
    matmuls act as PE filler inside the ACT-exp-bound attention window.
  - stat PSUM tiles live in the roomy psA2 pool, NOT the rope-rotation
    slot: a stat tile's lifetime is bound to the serial ACT Ln/Exp wall and
    would block every later-emitted rope use of a shared slot (-8us).
  - k-side rstd Ln/Exp pairs consolidated to 2 ops via column-packed PSUM
    tiles; softmax denominator handled per-(head,half): DVE reciprocal of
    the ones-row of the AV accumulator, PE ones-column outer product to
    broadcast it, normalization fused into the PSUM evacuation multiply.
  - PSUM bank choreography (8 banks): psA2(6: q01/k01+kstats) and psB(1:
    stats/rot/denom-po) release into psS(4 scores)+psAV(2)+psA1(1 filler)
    during attention; output blocks run after on 2 freed banks, split into
    864-column halves so the first half starts right as hf0 heads finish.
  - NOTE: gpsimd/InstISA ops (partition_broadcast etc.) fail codegen on this
    walrus build ("ISA wrong length"); single-partition APs must start at a
    32-aligned partition; ACT Rsqrt/Reciprocal are blocked by bass.
"""

import math

import numpy as np

import concourse.bass as bass
import concourse.mybir as mybir
import concourse.tile as tile
from concourse import bass_utils
from concourse.vector_clock import ScopedClock

F32 = mybir.dt.float32
BF16 = mybir.dt.bfloat16
AF = mybir.ActivationFunctionType
ALU = mybir.AluOpType

HID, HEADS, HD, MLP = 768, 12, 64, 3072
B, H, W, D = 2, 12, 12, 12
S = H * W * D  # 1728
ROT = 48
MAX_FREQ = 256.0
EPS_GN, EPS_LN = 1e-6, 1e-5

N_CORES = 8
TP = 4
HPC = 3  # heads per core
FFPC = 384  # ff pairs per core
# fused rows: [q0 q1 | k0 k1 | q2 pad64 | k2 pad64 | ffx(384) | gate(384)]
NFUSED = 4 * 128 + 2 * FFPC  # 1280
VCOLS = HPC * (HD + 1)  # 195: per head [v(64), one]

S_BLOCKS = [(0, 512), (512, 512), (1024, 448), (1472, 256)]
T_TILES = [(128 * j, 128) for j in range(13)] + [(1664, 64)]
HALF = S // 2  # 864
HSUBS = [(0, 512), (512, 352)]


class TileContextSplitDrain(tile.TileContext):
    """TileContext whose kernel-tail drain splits its semaphore waits across
    single-wait sync NOPs — the walrus build here rejects >2 sync waits on one
    SP CTRL instruction ("Too many sync wait commands")."""

    def _drain_and_barrier(self, tick_clock, wait_clock):
        probe = self.nc.sync.nop(nofuse=True)
        wait_clock.add_sem_waits(
            probe.ins, ScopedClock({None: tick_clock.global_clock})
        )
        si = probe.ins.sync_info
        waits = list(si.on_wait) if si is not None else []
        if si is not None:
            si.on_wait = waits[:1]
        for w in waits[1:]:
            n = self.nc.sync.nop(nofuse=True)
            nsi = n.ins.sync_info
            if nsi is None:
                n.ins.sync_info = mybir.SyncInfo(on_wait=[w], on_update=[])
            else:
                nsi.on_wait.append(w)
        self.nc.sync.drain()
        self.nc.all_engine_barrier()
        popped = self.nc._tile_sem_poison_stack.pop()
        assert popped is self._sem_poison
        self.nc.clear_and_free_semaphores(list(self.sems.allocated().values()))
        self.nc.all_engine_barrier()


def _split_excess_waits(nc, maxw=1):
    """walrus in this container caps sync waits per instruction; move extras
    onto preceding same-engine NOPs (waits execute in program order)."""
    nid = 0
    for bb in nc.m.functions[0].blocks:
        insts = bb.instructions
        i = 0
        while i < len(insts):
            inst = insts[i]
            si = inst.sync_info
            nw = len(si.on_wait) if si is not None and si.on_wait else 0
            if nw > maxw:
                waits = list(si.on_wait)
                si.on_wait = waits[-maxw:]
                extra = waits[:-maxw]
                pos = i
                for k in range(0, len(extra), maxw):
                    nop = mybir.InstNoOp(
                        name=f"I-waitsplit-{nid}", ins=[], outs=[]
                    )
                    nop.engine = inst.engine
                    nop.sync_info = mybir.SyncInfo(
                        on_wait=extra[k : k + maxw], on_update=[]
                    )
                    insts.insert(pos, nop)
                    nc.register_instruction(nop)
                    pos += 1
                    i += 1
                    nid += 1
            i += 1


def build_program():
    nc = bass.Bass(trn_type="TRN2")

    xT = nc.dram_tensor("xT", [HID, S], F32, kind="ExternalInput")
    wfT = nc.dram_tensor("wfT", [HID, NFUSED], BF16, kind="ExternalInput")
    # wv / wa / wff pre-flattened on host to single-DMA layouts
    wvT = nc.dram_tensor("wvT", [128, 6 * VCOLS], BF16, kind="ExternalInput")
    waT = nc.dram_tensor("waT", [HD, HPC * HID], BF16, kind="ExternalInput")
    wffT = nc.dram_tensor("wffT", [128, 3 * HID], BF16, kind="ExternalInput")
    cosT = nc.dram_tensor("cosT", [128, S], BF16, kind="ExternalInput")
    sinT = nc.dram_tensor("sinT", [128, S], BF16, kind="ExternalInput")
    # rr [128,128] with rr64 packed at rows 0:64, cols 128:192
    rrT = nc.dram_tensor("rrT", [128, 192], BF16, kind="ExternalInput")
    # nw cols 0:6, rmask cols 6:12
    nwrm = nc.dram_tensor("nwrm", [128, 12], F32, kind="ExternalInput")
    # wq01 cols 0:2, wk01 cols 2:4
    wqk01 = nc.dram_tensor("wqk01", [128, 4], BF16, kind="ExternalInput")
    # wq2 col 0, wk2 col 1
    wqk2 = nc.dram_tensor("wqk2", [HD, 2], BF16, kind="ExternalInput")
    eb2 = nc.dram_tensor("eb2", [2, 128], BF16, kind="ExternalInput")
    outT = nc.dram_tensor("outT", [HID, S], F32, kind="ExternalOutput")

    with TileContextSplitDrain(nc) as tc, nc.allow_low_precision(
        reason="bf16 activations; accumulation and stats stay fp32"
    ):
        with (
            tc.tile_pool(name="big", bufs=1) as pbg,
            tc.tile_pool(name="wts", bufs=1) as pwt,
            tc.tile_pool(name="scr", bufs=3) as pscr,
            tc.tile_pool(name="probs", bufs=16) as ppr,
            tc.tile_pool(name="outp", bufs=4) as pout,
            tc.tile_pool(name="small", bufs=1) as psm,
        ):
            # ---- persistent bf16 tiles -----------------------------------
            xn = [pbg.tile([128, S], BF16, name=f"xn{c}", tag=f"xn{c}")
                  for c in range(6)]
            ffa = [pbg.tile([128, S], BF16, name=f"ffa{i}", tag=f"ffa{i}")
                   for i in range(3)]
            vx = [pbg.tile([128, VCOLS], BF16, name=f"vx{j}", tag=f"vx{j}")
                  for j in range(14)]
            qab = pbg.tile([128, S], BF16, name="qab", tag="qab")
            kab = pbg.tile([128, S], BF16, name="kab", tag="kab")
            q2t = pbg.tile([HD, S], BF16, name="q2t", tag="q2t")
            k2t = pbg.tile([HD, S], BF16, name="k2t", tag="k2t")
            cosb = pbg.tile([128, S], BF16, name="cosb", tag="cosb")
            sinb = pbg.tile([128, S], BF16, name="sinb", tag="sinb")
            tsin = pbg.tile([128, S], BF16, name="tsin", tag="tsin")
            tcos = pbg.tile([128, S], BF16, name="tcos", tag="tcos")
            sqA = pbg.tile([128, S], BF16, name="sqA", tag="sqA")
            sqB = pbg.tile([HD, S], BF16, name="sqB", tag="sqB")
            att3 = [pbg.tile([HD, S], BF16, name=f"att{h}", tag=f"att{h}")
                    for h in range(3)]
            rqs = pbg.tile([2, 2 * S], BF16, name="rqs", tag="rqs")
            e2 = pbg.tile([2, 128], BF16, name="e2", tag="e2")
            e1b = pbg.tile([65, HD], BF16, name="e1b", tag="e1b")
            dsb = pbg.tile([65, HALF], BF16, name="dsb", tag="dsb")
            sqsc = pbg.tile([128, S], BF16, name="sqsc", tag="sqsc")

            rrm = pwt.tile([128, 192], BF16, name="rrm", tag="rrm")
            rr = rrm[:, 0:128]
            rr64 = rrm[0:HD, 128:192]
            wf = [pwt.tile([128, NFUSED], BF16, name=f"wf{c}", tag=f"wf{c}")
                  for c in range(6)]
            wvm = pwt.tile([128, 6 * VCOLS], BF16, name="wvm", tag="wvm")
            wam = pwt.tile([HD, HPC * HID], BF16, name="wam", tag="wam")
            wffm = pwt.tile([128, 3 * HID], BF16, name="wffm", tag="wffm")
            wqk01t = pwt.tile([128, 4], BF16, name="wqk01t", tag="wqk01t")
            wq01t = wqk01t[:, 0:2]
            wk01t = wqk01t[:, 2:4]
            wqk2t = pwt.tile([HD, 2], BF16, name="wqk2t", tag="wqk2t")
            wq2t = wqk2t[:, 0:1]
            wk2t = wqk2t[:, 1:2]

            nwrmt = psm.tile([128, 12], F32, name="nwrmt", tag="nwrmt")
            nwt = nwrmt[:, 0:6]
            rmk = nwrmt[:, 6:12]
            ss6 = psm.tile([128, 6], F32, name="ss6", tag="ss6")
            scale6 = psm.tile([128, 6], F32, name="scale6", tag="scale6")
            risc0 = psm.tile([128, 6], F32, name="risc0", tag="risc0")
            risc = psm.tile([128, 6], F32, name="risc", tag="risc")
            rk01 = psm.tile([128, 28], F32, name="rk01", tag="rk01")
            rk2 = psm.tile([128, 14], F32, name="rk2", tag="rk2")
            lnq = psm.tile([2, 512], F32, name="lnq", tag="lnq")
            cgn = psm.tile([128, 1], F32, name="cgn", tag="cgn")
            cln2 = psm.tile([2, 1], F32, name="cln2", tag="cln2")
            cln1 = psm.tile([1, 1], F32, name="cln1", tag="cln1")
            cl64 = psm.tile([128, 1], F32, name="cl64", tag="cl64")

            nc.vector.memset(e1b[64:65, :], 1.0)
            nc.vector.memset(cgn[:], EPS_GN)
            nc.vector.memset(cln2[:], EPS_LN)
            nc.vector.memset(cln1[:], EPS_LN)
            nc.vector.memset(cl64[:], 64.0 * EPS_LN)

            # tiny tables on the scalar DGE queue (keeps the SP queue free
            # for the x/wf stream); nwrm first — needed by chunk-0 stats
            nc.scalar.dma_start(nwrmt[:], nwrm[:])
            nc.scalar.dma_start(wqk01t[:], wqk01[:])
            nc.scalar.dma_start(wqk2t[:], wqk2[:])
            nc.scalar.dma_start(e2[:], eb2[:])
            nc.scalar.dma_start(rrm[:], rrT[:])

            with tc.tile_pool(name="xr", bufs=1) as pxr:
                xraw = [pxr.tile([128, S], F32, name=f"xr{c}", tag=f"xr{c}")
                        for c in range(6)]

                # interleaved x/wf chunk stream: the double-buffered q01/k01
                # PSUM groups pre-accumulate c-chunks as they land
                for c in range(6):
                    nc.sync.dma_start(xraw[c][:], xT[128 * c : 128 * (c + 1), :])
                    nc.sync.dma_start(wf[c][:], wfT[128 * c : 128 * (c + 1), :])
                nc.sync.dma_start(cosb[:], cosT[:])
                nc.sync.dma_start(sinb[:], sinT[:])
                nc.sync.dma_start(wvm[:], wvT[:])
                nc.sync.dma_start(wam[:], waT[:])
                nc.sync.dma_start(wffm[:], wffT[:])

                # ---- per-chunk RMSGroupNorm stats + normalized x ---------
                for c in range(6):
                    nc.scalar.activation(
                        sqsc[:],
                        xraw[c][:],
                        AF.Square,
                        accum_out=ss6[:, c : c + 1],
                    )
                    # std = sqrt(ss/S + eps); scale6 = nw/std; risc = std/nw
                    nc.scalar.activation(
                        ss6[:, c : c + 1], ss6[:, c : c + 1], AF.Sqrt,
                        bias=cgn[:], scale=1.0 / S,
                    )
                    nc.vector.reciprocal(
                        risc0[:, c : c + 1], ss6[:, c : c + 1]
                    )
                    nc.vector.tensor_mul(
                        scale6[:, c : c + 1], risc0[:, c : c + 1],
                        nwt[:, c : c + 1],
                    )
                    nc.vector.reciprocal(
                        risc0[:, c : c + 1], scale6[:, c : c + 1]
                    )
                    nc.vector.tensor_mul(
                        risc[:, c : c + 1], risc0[:, c : c + 1], rmk[:, c : c + 1]
                    )
                    nc.vector.tensor_scalar(
                        xn[c][:], xraw[c][:], scale6[:, c : c + 1], None, ALU.mult
                    )

            # ---- fused projection: q01, k01, q2, k2 ----------------------
            qk_dst = [qab, kab, q2t, k2t]
            with tc.tile_pool(name="psA2", bufs=6, space="PSUM") as psA2:
                for o in range(4):
                    for soff, slen in S_BLOCKS:
                        pt = psA2.tile([128, 512], F32, name="mm", tag="mm")
                        acc = pt[:, :slen]
                        for c in range(6):
                            nc.tensor.matmul(
                                acc,
                                wf[c][:, 128 * o : 128 * (o + 1)],
                                xn[c][:, soff : soff + slen],
                                start=(c == 0),
                                stop=(c == 5),
                            )
                        if o < 2:
                            if o == 0:
                                nc.vector.tensor_copy(
                                    qk_dst[o][:, soff : soff + slen], acc
                                )
                            else:
                                nc.scalar.activation(
                                    qk_dst[o][:, soff : soff + slen], acc,
                                    AF.Copy,
                                )
                        else:
                            nc.scalar.activation(
                                qk_dst[o][:, soff : soff + slen], acc[0:HD, :],
                                AF.Copy,
                            )

            with tc.tile_pool(name="psA1", bufs=1, space="PSUM") as psA1:
                # ---- v projection (token-major) --------------------------
                for j, (toff, tlen) in enumerate(T_TILES):
                    pt = psA1.tile([128, 512], F32, name="mmv", tag="mm")
                    acc = pt[:tlen, :VCOLS]
                    for c in range(6):
                        nc.tensor.matmul(
                            acc,
                            xn[c][:, toff : toff + tlen],
                            wv[c][:],
                            start=(c == 0),
                            stop=(c == 5),
                        )
                    nc.scalar.activation(vx[j][:tlen, :], acc, AF.Copy)
                    # ones columns for the softmax denominators
                    nc.vector.memset(vx[j][:tlen, HD : VCOLS : HD + 1], 1.0)

                # ---- q/k layernorm rstd stats ----------------------------
                with tc.tile_pool(name="psStat", bufs=1, space="PSUM") as psV:
                    # q side: rstd rows [2, S] (h0/h1) + [1, S] (h2);
                    # rsqrt as exp(-0.5 ln(var + eps))
                    nc.vector.tensor_mul(sqA[:], qab[:], qab[:])
                    for soff, slen in S_BLOCKS:
                        pt = psV.tile([2, 512], F32, name="vq", tag="vq")
                        nc.tensor.matmul(
                            pt[:, :slen], wq01t[:], sqA[:, soff : soff + slen]
                        )
                        nc.scalar.activation(
                            lnq[0:2, :slen], pt[:, :slen], AF.Ln, bias=cln2[:]
                        )
                        nc.scalar.activation(
                            rqs[0:2, soff : soff + slen], lnq[0:2, :slen],
                            AF.Exp, scale=-0.5,
                        )
                    nc.vector.tensor_mul(sqB[:], q2t[:], q2t[:])
                    for soff, slen in S_BLOCKS:
                        pt = psV.tile([2, 512], F32, name="vq2", tag="vq")
                        nc.tensor.matmul(
                            pt[0:1, :slen], wq2t[:], sqB[:, soff : soff + slen]
                        )
                        nc.scalar.activation(
                            lnq[0:1, :slen], pt[0:1, :slen], AF.Ln, bias=cln1[:]
                        )
                        nc.scalar.activation(
                            rqs[0:1, S + soff : S + soff + slen],
                            lnq[0:1, :slen],
                            AF.Exp, scale=-0.5,
                        )

                    # k side: rstd/8 columns, consolidated Rsqrts
                    nc.vector.tensor_mul(sqA[:], kab[:], kab[:])
                    nc.vector.tensor_mul(sqB[:], k2t[:], k2t[:])
                    vst = psV.tile([128, 48], F32, name="vst", tag="vst")
                    nc.vector.memset(vst[:], 1.0)
                    for j, (toff, tlen) in enumerate(T_TILES):
                        nc.tensor.matmul(
                            vst[:tlen, 2 * j : 2 * j + 2],
                            sqA[:, toff : toff + tlen],
                            wk01t[:],
                        )
                        nc.tensor.matmul(
                            vst[:tlen, 28 + j : 29 + j],
                            sqB[:, toff : toff + tlen],
                            wk2t[:],
                        )
                    nc.scalar.activation(
                        rk01[:], vst[:, 0:28], AF.Ln, bias=cl64[:]
                    )
                    nc.scalar.activation(
                        rk01[:], rk01[:], AF.Exp, scale=-0.5
                    )
                    nc.scalar.activation(
                        rk2[:], vst[:, 28:42], AF.Ln, bias=cl64[:]
                    )
                    nc.scalar.activation(
                        rk2[:], rk2[:], AF.Exp, scale=-0.5
                    )

                # rstd_q broadcasts on the idle GPSIMD engine
                nc.gpsimd.partition_broadcast(qrs_b[0:HD, :], rqs[0:1, 0:S])
                nc.gpsimd.partition_broadcast(qrs_b[HD:128, :], rqs[1:2, 0:S])
                nc.gpsimd.partition_broadcast(q2rs_b[:], rqs[0:1, S : 2 * S])

                with tc.tile_pool(name="psB", bufs=1, space="PSUM") as psB:

                    def rope(dst, nrow, rmat, rs_b):
                        for soff, slen in S_BLOCKS:
                            pt = psB.tile([128, 512], F32, name="rot", tag="rot")
                            nc.tensor.matmul(
                                pt[:nrow, :slen],
                                rmat[:],
                                dst[:, soff : soff + slen],
                            )
                            nc.vector.tensor_mul(
                                tsin[:nrow, soff : soff + slen],
                                pt[:nrow, :slen],
                                sinb[:nrow, soff : soff + slen],
                            )
                        nc.vector.tensor_mul(
                            tcos[:nrow, :], dst[:], cosb[:nrow, :]
                        )
                        if rs_b is None:
                            nc.vector.tensor_add(
                                dst[:], tsin[:nrow, :], tcos[:nrow, :]
                            )
                        else:
                            nc.vector.tensor_add(
                                tsin[:nrow, :], tsin[:nrow, :], tcos[:nrow, :]
                            )
                            nc.vector.tensor_mul(
                                dst[:], tsin[:nrow, :], rs_b[:]
                            )

                    rope(qab, 128, rr, qrs_b)
                    rope(kab, 128, rr, None)
                    rope(q2t, HD, rr64, q2rs_b)
                    rope(k2t, HD, rr64, None)

                    # ---- attention ---------------------------------------
                    qsl = [qab[0:HD], qab[HD:128], q2t[:]]
                    ksl = [kab[0:HD], kab[HD:128], k2t[:]]

                    with (
                        tc.tile_pool(name="psS", bufs=2, space="PSUM") as psS,
                        tc.tile_pool(name="psAV", bufs=1, space="PSUM") as psAV,
                    ):
                        for hf in range(2):
                            for h in range(HPC):
                                hoff = HALF * hf
                                av = psAV.tile(
                                    [65, HALF], F32, name="av", tag="av"
                                )
                                for j, (toff, tlen) in enumerate(T_TILES):
                                    pb = ppr.tile(
                                        [128, HALF], BF16, name="pb", tag="pb"
                                    )
                                    sc = psS.tile(
                                        [128, HALF], F32, name="sc", tag="sc"
                                    )
                                    for aoff, alen in HSUBS:
                                        nc.tensor.matmul(
                                            sc[:tlen, aoff : aoff + alen],
                                            ksl[h][:, toff : toff + tlen],
                                            qsl[h][
                                                :, hoff + aoff : hoff + aoff + alen
                                            ],
                                        )
                                    scl = (
                                        rk01[:tlen, 2 * j + h : 2 * j + h + 1]
                                        if h < 2
                                        else rk2[:tlen, j : j + 1]
                                    )
                                    nc.scalar.activation(
                                        pb[:tlen, :], sc[:tlen, :], AF.Exp,
                                        scale=scl,
                                    )
                                    for aoff, alen in HSUBS:
                                        nc.tensor.matmul(
                                            av[:, aoff : aoff + alen],
                                            vx[j][
                                                :tlen,
                                                (HD + 1) * h : (HD + 1) * (h + 1),
                                            ],
                                            pb[:tlen, aoff : aoff + alen],
                                            start=(j == 0),
                                            stop=(j == 13),
                                        )
                                # normalize: recip of denominator row, GPSIMD
                                # broadcast, single fused evac multiply
                                nc.vector.reciprocal(
                                    dsb[64:65, :], av[64:65, :]
                                )
                                for aoff, alen in HSUBS:
                                    nc.vector.tensor_copy(
                                        att3[h][
                                            :, hoff + aoff : hoff + aoff + alen
                                        ],
                                        av[0:HD, aoff : aoff + alen],
                                    )
                                for aoff, alen in HSUBS:
                                    po = psS.tile(
                                        [128, 512], F32, name="pod", tag="sc"
                                    )
                                    nc.tensor.matmul(
                                        po[0:HD, :alen],
                                        e1b[64:65, :],
                                        dsb[64:65, aoff : aoff + alen],
                                    )
                                    nc.vector.tensor_mul(
                                        att3[h][
                                            :, hoff + aoff : hoff + aoff + alen
                                        ],
                                        att3[h][
                                            :, hoff + aoff : hoff + aoff + alen
                                        ],
                                        po[0:HD, :alen],
                                    )

                    # ---- ffx/gate fused blocks + output blocks, both on
                    # the psA1 slot (PE filler during the ACT-bound attention
                    # window; output halves start as soon as the matching
                    # attention halves and ff columns are done) -------------
                    def filler_block(soff, slen):
                        for o in range(4, 10):
                            pt = psA1.tile([128, 512], F32, name="mmf", tag="mm")
                            acc = pt[:, :slen]
                            for c in range(6):
                                nc.tensor.matmul(
                                    acc,
                                    wf[c][:, 128 * o : 128 * (o + 1)],
                                    xn[c][:, soff : soff + slen],
                                    start=(c == 0),
                                    stop=(c == 5),
                                )
                            if o < 7:
                                nc.vector.tensor_copy(
                                    ffa[o - 4][:, soff : soff + slen], acc
                                )
                            else:
                                gs = pscr.tile(
                                    [128, 512], BF16, name="gs", tag="gs"
                                )
                                nc.scalar.activation(
                                    gs[:, :slen], acc, AF.Silu
                                )
                                nc.vector.tensor_mul(
                                    ffa[o - 7][:, soff : soff + slen],
                                    ffa[o - 7][:, soff : soff + slen],
                                    gs[:, :slen],
                                )

                    def out_block(soff, slen, pools):
                        for o in range(6):
                            pool = pools[o % len(pools)]
                            pt = pool.tile([128, 512], F32, name="oc", tag="oc")
                            acc = pt[:, :slen]
                            for h in range(3):
                                nc.tensor.matmul(
                                    acc,
                                    wam[
                                        :,
                                        HID * h + 128 * o : HID * h + 128 * (o + 1),
                                    ],
                                    att3[h][:, soff : soff + slen],
                                    start=(h == 0),
                                    stop=False,
                                )
                            for c in range(3):
                                nc.tensor.matmul(
                                    acc,
                                    wffm[
                                        :,
                                        HID * c + 128 * o : HID * c + 128 * (o + 1),
                                    ],
                                    ffa[c][:, soff : soff + slen],
                                    start=False,
                                    stop=(c == 2),
                                )
                            ob = pout.tile(
                                [128, 512], F32, name="obt", tag="obt"
                            )
                            nc.vector.scalar_tensor_tensor(
                                ob[:, :slen],
                                xn[o][:, soff : soff + slen],
                                risc[:, o : o + 1],
                                acc,
                                ALU.mult,
                                ALU.add,
                            )
                            nc.sync.dma_start(
                                outT[128 * o : 128 * (o + 1), soff : soff + slen],
                                ob[:, :slen],
                            )

                    filler_block(*S_BLOCKS[0])
                    filler_block(*S_BLOCKS[1])
                    filler_block(*S_BLOCKS[2])
                    filler_block(*S_BLOCKS[3])
                    psB.release()
                    psC = tc.alloc_tile_pool(name="psC", bufs=4, space="PSUM")
                    for soff, slen in HALVES[0] + HALVES[1]:
                        out_block(soff, slen, [psC])
                    psC.release()
                    psA1.release()
    _split_excess_waits(nc)
    return nc


# ---------------------------------------------------------------------------
# host-side preparation
# ---------------------------------------------------------------------------


def _axial_freqs():
    base = np.linspace(1.0, MAX_FREQ / 2, 8) * math.pi

    def ax(n):
        pos = np.linspace(-1.0, 1.0, n)
        return np.repeat(pos[:, None] * base[None, :], 2, axis=-1)

    fH = np.broadcast_to(ax(H)[:, None, None, :], (H, W, D, 16))
    fW = np.broadcast_to(ax(W)[None, :, None, :], (H, W, D, 16))
    fD = np.broadcast_to(ax(D)[None, None, :, :], (H, W, D, 16))
    return np.concatenate((fH, fW, fD), axis=-1).reshape(S, ROT)


def _bf16(a):
    import ml_dtypes

    return np.ascontiguousarray(np.asarray(a, np.float32)).astype(
        ml_dtypes.bfloat16
    )


def _prep_core_inputs(x, norm1_w, w_fused, b_fused, q_gamma, q_beta, k_gamma,
                      k_beta, w_attn, w_ff, b_ff):
    """Returns list of 8 in_maps (core = b*4 + r)."""
    f64 = np.float64
    w_fused = np.asarray(w_fused, f64)
    q_gamma = np.asarray(q_gamma, f64)
    k_gamma = np.asarray(k_gamma, f64)

    if np.any(np.asarray(b_fused)) or np.any(np.asarray(b_ff)):
        raise NotImplementedError("nonzero biases not supported by this kernel")
    if np.any(np.asarray(q_beta)) or np.any(np.asarray(k_beta)):
        raise NotImplementedError("nonzero q/k beta not supported by this kernel")
    if np.any(q_gamma == 0) or np.any(k_gamma == 0):
        raise NotImplementedError("zero gamma not supported by this kernel")

    M = np.eye(HD) - np.ones((HD, HD)) / HD
    Aq = np.diag(q_gamma) @ M
    Ak = np.diag(k_gamma) @ M
    R = np.zeros((HD, HD))
    for i in range(ROT // 2):
        R[2 * i, 2 * i + 1] = -1.0
        R[2 * i + 1, 2 * i] = 1.0
    R2 = np.zeros((128, 128))
    R2[0:64, 0:64] = R
    R2[64:128, 64:128] = R

    freqs = _axial_freqs()
    cos64 = np.ones((HD, S))
    sin64 = np.zeros((HD, S))
    cos64[:ROT, :] = np.cos(freqs).T
    sin64[:ROT, :] = np.sin(freqs).T
    cosT = _bf16(np.vstack([cos64, cos64]))
    sinT = _bf16(np.vstack([sin64, sin64]))

    wq_full = w_fused[MLP : MLP + HID]
    wk_full = w_fused[MLP + HID : MLP + 2 * HID]
    wv_full = w_fused[MLP + 2 * HID :]
    ffx_full = w_fused[: MLP // 2]
    gate_full = w_fused[MLP // 2 : MLP]

    nw = np.asarray(norm1_w, np.float32).reshape(6, 128).T
    wqk01 = np.zeros((128, 4))
    wqk01[0:64, 0] = 1.0 / (HD * q_gamma**2)
    wqk01[64:128, 1] = 1.0 / (HD * q_gamma**2)
    wqk01[0:64, 2] = 1.0 / k_gamma**2
    wqk01[64:128, 3] = 1.0 / k_gamma**2
    wqk2 = np.zeros((HD, 2))
    wqk2[:, 0] = 1.0 / (HD * q_gamma**2)
    wqk2[:, 1] = 1.0 / k_gamma**2
    rrm = np.zeros((128, 192))
    rrm[:, 0:128] = R2.T
    rrm[0:64, 128:192] = R.T
    eb2_np = np.zeros((2, 128))
    eb2_np[0, 0:64] = 1.0
    eb2_np[1, 64:128] = 1.0

    pad = np.zeros((64, HID))
    in_maps = []
    for core in range(N_CORES):
        b, r = divmod(core, TP)
        hs = [HPC * r + i for i in range(HPC)]
        q3 = [Aq @ wq_full[HD * h : HD * (h + 1)] for h in hs]
        k3 = [Ak @ wk_full[HD * h : HD * (h + 1)] for h in hs]
        ffx = ffx_full[FFPC * r : FFPC * (r + 1)]
        gate = gate_full[FFPC * r : FFPC * (r + 1)]
        wfT_np = np.vstack(
            [q3[0], q3[1], k3[0], k3[1], q3[2], pad, k3[2], pad, ffx, gate]
        ).T
        wv_mat = np.zeros((VCOLS, HID))
        for i, h in enumerate(hs):
            wv_mat[(HD + 1) * i : (HD + 1) * i + HD] = wv_full[HD * h : HD * (h + 1)]
        # [768, VCOLS] -> [128, 6*VCOLS] (chunk-c columns side by side)
        wvT_np = (
            wv_mat.T.reshape(6, 128, VCOLS)
            .transpose(1, 0, 2)
            .reshape(128, 6 * VCOLS)
        )
        acols = np.concatenate([np.arange(HD * h, HD * (h + 1)) for h in hs])
        waT_np = (
            np.asarray(w_attn, f64)[:, acols]
            .T.reshape(3, HD, HID)
            .transpose(1, 0, 2)
            .reshape(HD, 3 * HID)
        )
        wffT_np = (
            np.asarray(w_ff, f64)[:, FFPC * r : FFPC * (r + 1)]
            .T.reshape(3, 128, HID)
            .transpose(1, 0, 2)
            .reshape(128, 3 * HID)
        )
        nwrm = np.zeros((128, 12), np.float32)
        nwrm[:, 0:6] = nw
        nwrm[:, 6:12] = 1.0 if r == 0 else 0.0
        in_maps.append(
            {
                "xT": np.ascontiguousarray(
                    np.asarray(x[b], np.float32).reshape(HID, S)
                ),
                "wfT": _bf16(wfT_np),
                "wvT": _bf16(wvT_np),
                "waT": _bf16(waT_np),
                "wffT": _bf16(wffT_np),
                "cosT": cosT,
                "sinT": sinT,
                "rrT": _bf16(rrm),
                "nwrm": nwrm,
                "wqk01": _bf16(wqk01),
                "wqk2": _bf16(wqk2),
                "eb2": _bf16(eb2_np),
            }
        )
    return in_maps


_NC_CACHE = {}


def get_program():
    if "nc" not in _NC_CACHE:
        _NC_CACHE["nc"] = build_program()
    return _NC_CACHE["nc"]


def kernel(**inputs) -> np.ndarray:
    nc = get_program()
    in_maps = _prep_core_inputs(**inputs)
    res = bass_utils.run_bass_kernel_spmd(nc, in_maps, core_ids=list(range(N_CORES)))
    out = np.zeros((B, HID, H, W, D), np.float32)
    for core in range(N_CORES):
        b = core // TP
        out[b] += res.results[core]["outT"].reshape(HID, H, W, D)
    return out


# revision 57
# speedup vs baseline: 1.0714x; 1.0714x over previous
"""Trainium2 Bass kernel for nn_FullAttention_17789754540074.

Self-contained: takes the FULL inputs of reference.setup_inputs(), returns the
FULL output. Internally shards across 8 NeuronCores as 2-way data parallel
(batch) x 4-way tensor parallel (3 heads + 384 FF pairs per rank), runs one
SPMD Bass/Tile program via run_bass_kernel_spmd, and sums the 4 partial
outputs per batch on the host (the unshard step for partial-sum TP sharding).

Design notes (vs the earlier fp32r version; 265.5us -> 185.0us sim time):
  - bf16 activations/weights everywhere off the critical-precision path; x,
    PSUM accumulation, RMS stats, rk/rq rstds and the residual merge stay
    fp32 (hw rel err 1.8e-3).
  - per-channel-chunk RMS stats + interleaved x/wf chunk DMAs; aux tables
    ride the ACT DGE queue so the SP queue streams x/wf back-to-back.
  - single flat SBUF scope with emission order = scheduler priority:
    q01/k01 blocks -> their stats -> half-granular ropes -> q2/k2 -> v ->
    attention (hf-major) -> ffx/gate blocks -> output blocks.  The ffx/gate
    matmuls act as PE filler inside the ACT-exp-bound attention window.
  - stat PSUM tiles live in the roomy psA2 pool, NOT the rope-rotation
    slot: a stat tile's lifetime is bound to the serial ACT Ln/Exp wall and
    would block every later-emitted rope use of a shared slot (-8us).
  - k-side rstd Ln/Exp pairs consolidated to 2 ops via column-packed PSUM
    tiles; softmax denominator handled per-(head,half): DVE reciprocal of
    the ones-row of the AV accumulator, PE ones-column outer product to
    broadcast it, normalization fused into the PSUM evacuation multiply.
  - PSUM bank choreography (8 banks): psA2(6: q01/k01+kstats) and psB(1:
    stats/rot/denom-po) release into psS(4 scores)+psAV(2)+psA1(1 filler)
    during attention; output blocks run after on 2 freed banks, split into
    864-column halves so the first half starts right as hf0 heads finish.
  - NOTE: gpsimd/InstISA ops (partition_broadcast etc.) fail codegen on this
    walrus build ("ISA wrong length"); single-partition APs must start at a
    32-aligned partition; ACT Rsqrt/Reciprocal are blocked by bass.
"""

import math

import numpy as np

import concourse.bass as bass
import concourse.mybir as mybir
import concourse.tile as tile
from concourse import bass_utils
from concourse.vector_clock import ScopedClock

F32 = mybir.dt.float32
BF16 = mybir.dt.bfloat16
AF = mybir.ActivationFunctionType
ALU = mybir.AluOpType

HID, HEADS, HD, MLP = 768, 12, 64, 3072
B, H, W, D = 2, 12, 12, 12
S = H * W * D  # 1728
ROT = 48
MAX_FREQ = 256.0
EPS_GN, EPS_LN = 1e-6, 1e-5

N_CORES = 8
TP = 4
HPC = 3  # heads per core
FFPC = 384  # ff pairs per core
# fused rows: [q0 q1 | k0 k1 | q2 pad64 | k2 pad64 | ffx(384) | gate(384)]
NFUSED = 4 * 128 + 2 * FFPC  # 1280
VCOLS = HPC * (HD + 1)  # 195: per head [v(64), one]

S_BLOCKS = [(0, 512), (512, 512), (1024, 448), (1472, 256)]
T_TILES = [(128 * j, 128) for j in range(13)] + [(1664, 64)]
HALF = S // 2  # 864
HSUBS = [(0, 512), (512, 352)]


class TileContextSplitDrain(tile.TileContext):
    """TileContext whose kernel-tail drain splits its semaphore waits across
    single-wait sync NOPs — the walrus build here rejects >2 sync waits on one
    SP CTRL instruction ("Too many sync wait commands")."""

    def _drain_and_barrier(self, tick_clock, wait_clock):
        probe = self.nc.sync.nop(nofuse=True)
        wait_clock.add_sem_waits(
            probe.ins, ScopedClock({None: tick_clock.global_clock})
        )
        si = probe.ins.sync_info
        waits = list(si.on_wait) if si is not None else []
        if si is not None:
            si.on_wait = waits[:1]
        for w in waits[1:]:
            n = self.nc.sync.nop(nofuse=True)
            nsi = n.ins.sync_info
            if nsi is None:
                n.ins.sync_info = mybir.SyncInfo(on_wait=[w], on_update=[])
            else:
                nsi.on_wait.append(w)
        self.nc.sync.drain()
        self.nc.all_engine_barrier()
        popped = self.nc._tile_sem_poison_stack.pop()
        assert popped is self._sem_poison
        self.nc.clear_and_free_semaphores(list(self.sems.allocated().values()))
        self.nc.all_engine_barrier()


def _split_excess_waits(nc, maxw=1):
    """walrus in this container caps sync waits per instruction; move extras
    onto preceding same-engine NOPs (waits execute in program order)."""
    nid = 0
    for bb in nc.m.functions[0].blocks:
        insts = bb.instructions
        i = 0
        while i < len(insts):
            inst = insts[i]
            si = inst.sync_info
            nw = len(si.on_wait) if si is not None and si.on_wait else 0
            if nw > maxw:
                waits = list(si.on_wait)
                si.on_wait = waits[-maxw:]
                extra = waits[:-maxw]
                pos = i
                for k in range(0, len(extra), maxw):
                    nop = mybir.InstNoOp(
                        name=f"I-waitsplit-{nid}", ins=[], outs=[]
                    )
                    nop.engine = inst.engine
                    nop.sync_info = mybir.SyncInfo(
                        on_wait=extra[k : k + maxw], on_update=[]
                    )
                    insts.insert(pos, nop)
                    nc.register_instruction(nop)
                    pos += 1
                    i += 1
                    nid += 1
            i += 1


def build_program():
    nc = bass.Bass(trn_type="TRN2")

    xT = nc.dram_tensor("xT", [HID, S], F32, kind="ExternalInput")
    wfT = nc.dram_tensor("wfT", [HID, NFUSED], BF16, kind="ExternalInput")
    # wv / wa / wff pre-flattened on host to single-DMA layouts
    wvT = nc.dram_tensor("wvT", [128, 6 * VCOLS], BF16, kind="ExternalInput")
    waT = nc.dram_tensor("waT", [HD, HPC * HID], BF16, kind="ExternalInput")
    wffT = nc.dram_tensor("wffT", [128, 3 * HID], BF16, kind="ExternalInput")
    cosT = nc.dram_tensor("cosT", [128, S], BF16, kind="ExternalInput")
    sinT = nc.dram_tensor("sinT", [128, S], BF16, kind="ExternalInput")
    # rr [128,128] with rr64 packed at rows 0:64, cols 128:192
    rrT = nc.dram_tensor("rrT", [128, 192], BF16, kind="ExternalInput")
    # nw cols 0:6, rmask cols 6:12
    nwrm = nc.dram_tensor("nwrm", [128, 12], F32, kind="ExternalInput")
    # wq01 cols 0:2, wk01 cols 2:4
    wqk01 = nc.dram_tensor("wqk01", [128, 4], BF16, kind="ExternalInput")
    # wq2 col 0, wk2 col 1
    wqk2 = nc.dram_tensor("wqk2", [HD, 2], BF16, kind="ExternalInput")
    eb2 = nc.dram_tensor("eb2", [2, 128], BF16, kind="ExternalInput")
    outT = nc.dram_tensor("outT", [HID, S], F32, kind="ExternalOutput")

    with TileContextSplitDrain(nc) as tc, nc.allow_low_precision(
        reason="bf16 activations; accumulation and stats stay fp32"
    ):
        with (
            tc.tile_pool(name="big", bufs=1) as pbg,
            tc.tile_pool(name="wts", bufs=1) as pwt,
            tc.tile_pool(name="scr", bufs=3) as pscr,
            tc.tile_pool(name="probs", bufs=16) as ppr,
            tc.tile_pool(name="outp", bufs=4) as pout,
            tc.tile_pool(name="small", bufs=1) as psm,
        ):
            # ---- persistent bf16 tiles -----------------------------------
            xn = [pbg.tile([128, S], BF16, name=f"xn{c}", tag=f"xn{c}")
                  for c in range(6)]
            ffa = [pbg.tile([128, S], BF16, name=f"ffa{i}", tag=f"ffa{i}")
                   for i in range(3)]
            vx = [pbg.tile([128, VCOLS], BF16, name=f"vx{j}", tag=f"vx{j}")
                  for j in range(14)]
            qab = pbg.tile([128, S], BF16, name="qab", tag="qab")
            kab = pbg.tile([128, S], BF16, name="kab", tag="kab")
            q2t = pbg.tile([HD, S], BF16, name="q2t", tag="q2t")
            k2t = pbg.tile([HD, S], BF16, name="k2t", tag="k2t")
            cosb = pbg.tile([128, S], BF16, name="cosb", tag="cosb")
            sinb = pbg.tile([128, S], BF16, name="sinb", tag="sinb")
            tsin = pbg.tile([128, S], BF16, name="tsin", tag="tsin")
            tcos = pbg.tile([128, S], BF16, name="tcos", tag="tcos")
            sqA = pbg.tile([128, S], BF16, name="sqA", tag="sqA")
            sqB = pbg.tile([HD, S], BF16, name="sqB", tag="sqB")
            att3 = [pbg.tile([HD, S], BF16, name=f"att{h}", tag=f"att{h}")
                    for h in range(3)]
            rqs = pbg.tile([2, 2 * S], BF16, name="rqs", tag="rqs")
            e2 = pbg.tile([2, 128], BF16, name="e2", tag="e2")
            e1b = pbg.tile([65, HD], BF16, name="e1b", tag="e1b")
            dsb = pbg.tile([65, HALF], BF16, name="dsb", tag="dsb")
            sqsc = pbg.tile([128, S], BF16, name="sqsc", tag="sqsc")

            rrm = pwt.tile([128, 192], BF16, name="rrm", tag="rrm")
            rr = rrm[:, 0:128]
            rr64 = rrm[0:HD, 128:192]
            wf = [pwt.tile([128, NFUSED], BF16, name=f"wf{c}", tag=f"wf{c}")
                  for c in range(6)]
            wvm = pwt.tile([128, 6 * VCOLS], BF16, name="wvm", tag="wvm")
            wam = pwt.tile([HD, HPC * HID], BF16, name="wam", tag="wam")
            wffm = pwt.tile([128, 3 * HID], BF16, name="wffm", tag="wffm")
            wqk01t = pwt.tile([128, 4], BF16, name="wqk01t", tag="wqk01t")
            wq01t = wqk01t[:, 0:2]
            wk01t = wqk01t[:, 2:4]
            wqk2t = pwt.tile([HD, 2], BF16, name="wqk2t", tag="wqk2t")
            wq2t = wqk2t[:, 0:1]
            wk2t = wqk2t[:, 1:2]

            nwrmt = psm.tile([128, 12], F32, name="nwrmt", tag="nwrmt")
            nwt = nwrmt[:, 0:6]
            rmk = nwrmt[:, 6:12]
            ss6 = psm.tile([128, 6], F32, name="ss6", tag="ss6")
            scale6 = psm.tile([128, 6], F32, name="scale6", tag="scale6")
            risc0 = psm.tile([128, 6], F32, name="risc0", tag="risc0")
            risc = psm.tile([128, 6], F32, name="risc", tag="risc")
            rk01 = psm.tile([128, 28], F32, name="rk01", tag="rk01")
            rk2 = psm.tile([128, 14], F32, name="rk2", tag="rk2")
            lnq = psm.tile([2, 512], F32, name="lnq", tag="lnq")
            cgn = psm.tile([128, 1], F32, name="cgn", tag="cgn")
            cln2 = psm.tile([2, 1], F32, name="cln2", tag="cln2")
            cln1 = psm.tile([1, 1], F32, name="cln1", tag="cln1")
            cl64 = psm.tile([128, 1], F32, name="cl64", tag="cl64")

            nc.vector.memset(e1b[64:65, :], 1.0)
            nc.vector.memset(cgn[:], EPS_GN)
            nc.vector.memset(cln2[:], EPS_LN)
            nc.vector.memset(cln1[:], EPS_LN)
            nc.vector.memset(cl64[:], 64.0 * EPS_LN)

            # tiny tables on the scalar DGE queue (keeps the SP queue free
            # for the x/wf stream); nwrm first — needed by chunk-0 stats
            nc.scalar.dma_start(nwrmt[:], nwrm[:])
            nc.scalar.dma_start(wqk01t[:], wqk01[:])
            nc.scalar.dma_start(wqk2t[:], wqk2[:])
            nc.scalar.dma_start(e2[:], eb2[:])
            nc.scalar.dma_start(rrm[:], rrT[:])

            with tc.tile_pool(name="xr", bufs=1) as pxr:
                xraw = [pxr.tile([128, S], F32, name=f"xr{c}", tag=f"xr{c}")
                        for c in range(6)]

                # interleaved x/wf chunk stream: the double-buffered q01/k01
                # PSUM groups pre-accumulate c-chunks as they land
                for c in range(6):
                    nc.sync.dma_start(xraw[c][:], xT[128 * c : 128 * (c + 1), :])
                    nc.sync.dma_start(wf[c][:], wfT[128 * c : 128 * (c + 1), :])
                nc.sync.dma_start(cosb[:], cosT[:])
                nc.sync.dma_start(sinb[:], sinT[:])
                nc.sync.dma_start(wvm[:], wvT[:])
                nc.sync.dma_start(wam[:], waT[:])
                nc.sync.dma_start(wffm[:], wffT[:])

                # ---- per-chunk RMSGroupNorm stats + normalized x ---------
                for c in range(6):
                    nc.scalar.activation(
                        sqsc[:],
                        xraw[c][:],
                        AF.Square,
                        accum_out=ss6[:, c : c + 1],
                    )
                    # std = sqrt(ss/S + eps); scale6 = nw/std; risc = std/nw
                    nc.scalar.activation(
                        ss6[:, c : c + 1], ss6[:, c : c + 1], AF.Sqrt,
                        bias=cgn[:], scale=1.0 / S,
                    )
                    nc.vector.reciprocal(
                        risc0[:, c : c + 1], ss6[:, c : c + 1]
                    )
                    nc.vector.tensor_mul(
                        scale6[:, c : c + 1], risc0[:, c : c + 1],
                        nwt[:, c : c + 1],
                    )
                    nc.vector.reciprocal(
                        risc0[:, c : c + 1], scale6[:, c : c + 1]
                    )
                    nc.vector.tensor_mul(
                        risc[:, c : c + 1], risc0[:, c : c + 1], rmk[:, c : c + 1]
                    )
                    nc.vector.tensor_scalar(
                        xn[c][:], xraw[c][:], scale6[:, c : c + 1], None, ALU.mult
                    )

            # ---- fused projection: q01, k01, q2, k2 ----------------------
            qk_dst = [qab, kab, q2t, k2t]
            with tc.tile_pool(name="psA2", bufs=6, space="PSUM") as psA2:
                for o in range(4):
                    for soff, slen in S_BLOCKS:
                        pt = psA2.tile([128, 512], F32, name="mm", tag="mm")
                        acc = pt[:, :slen]
                        for c in range(6):
                            nc.tensor.matmul(
                                acc,
                                wf[c][:, 128 * o : 128 * (o + 1)],
                                xn[c][:, soff : soff + slen],
                                start=(c == 0),
                                stop=(c == 5),
                            )
                        if o < 2:
                            if o == 0:
                                nc.vector.tensor_copy(
                                    qk_dst[o][:, soff : soff + slen], acc
                                )
                            else:
                                nc.scalar.activation(
                                    qk_dst[o][:, soff : soff + slen], acc,
                                    AF.Copy,
                                )
                        else:
                            nc.scalar.activation(
                                qk_dst[o][:, soff : soff + slen], acc[0:HD, :],
                                AF.Copy,
                            )

            with tc.tile_pool(name="psA1", bufs=1, space="PSUM") as psA1:
                # ---- v projection (token-major) --------------------------
                for j, (toff, tlen) in enumerate(T_TILES):
                    pt = psA1.tile([128, 512], F32, name="mmv", tag="mm")
                    acc = pt[:tlen, :VCOLS]
                    for c in range(6):
                        nc.tensor.matmul(
                            acc,
                            xn[c][:, toff : toff + tlen],
                            wv[c][:],
                            start=(c == 0),
                            stop=(c == 5),
                        )
                    nc.scalar.activation(vx[j][:tlen, :], acc, AF.Copy)
                    # ones columns for the softmax denominators
                    nc.vector.memset(vx[j][:tlen, HD : VCOLS : HD + 1], 1.0)

                # ---- q/k layernorm rstd stats ----------------------------
                with tc.tile_pool(name="psStat", bufs=1, space="PSUM") as psV:
                    # q side: rstd rows [2, S] (h0/h1) + [1, S] (h2);
                    # rsqrt as exp(-0.5 ln(var + eps))
                    nc.vector.tensor_mul(sqA[:], qab[:], qab[:])
                    for soff, slen in S_BLOCKS:
                        pt = psV.tile([2, 512], F32, name="vq", tag="vq")
                        nc.tensor.matmul(
                            pt[:, :slen], wq01t[:], sqA[:, soff : soff + slen]
                        )
                        nc.scalar.activation(
                            lnq[0:2, :slen], pt[:, :slen], AF.Ln, bias=cln2[:]
                        )
                        nc.scalar.activation(
                            rqs[0:2, soff : soff + slen], lnq[0:2, :slen],
                            AF.Exp, scale=-0.5,
                        )
                    nc.vector.tensor_mul(sqB[:], q2t[:], q2t[:])
                    for soff, slen in S_BLOCKS:
                        pt = psV.tile([2, 512], F32, name="vq2", tag="vq")
                        nc.tensor.matmul(
                            pt[0:1, :slen], wq2t[:], sqB[:, soff : soff + slen]
                        )
                        nc.scalar.activation(
                            lnq[0:1, :slen], pt[0:1, :slen], AF.Ln, bias=cln1[:]
                        )
                        nc.scalar.activation(
                            rqs[0:1, S + soff : S + soff + slen],
                            lnq[0:1, :slen],
                            AF.Exp, scale=-0.5,
                        )

                    # k side: rstd/8 columns, consolidated Rsqrts
                    nc.vector.tensor_mul(sqA[:], kab[:], kab[:])
                    nc.vector.tensor_mul(sqB[:], k2t[:], k2t[:])
                    vst = psV.tile([128, 48], F32, name="vst", tag="vst")
                    nc.vector.memset(vst[:], 1.0)
                    for j, (toff, tlen) in enumerate(T_TILES):
                        nc.tensor.matmul(
                            vst[:tlen, 2 * j : 2 * j + 2],
                            sqA[:, toff : toff + tlen],
                            wk01t[:],
                        )
                        nc.tensor.matmul(
                            vst[:tlen, 28 + j : 29 + j],
                            sqB[:, toff : toff + tlen],
                            wk2t[:],
                        )
                    nc.scalar.activation(
                        rk01[:], vst[:, 0:28], AF.Ln, bias=cl64[:]
                    )
                    nc.scalar.activation(
                        rk01[:], rk01[:], AF.Exp, scale=-0.5
                    )
                    nc.scalar.activation(
                        rk2[:], vst[:, 28:42], AF.Ln, bias=cl64[:]
                    )
                    nc.scalar.activation(
                        rk2[:], rk2[:], AF.Exp, scale=-0.5
                    )

                # rstd_q broadcasts on the idle GPSIMD engine
                nc.gpsimd.partition_broadcast(qrs_b[0:HD, :], rqs[0:1, 0:S])
                nc.gpsimd.partition_broadcast(qrs_b[HD:128, :], rqs[1:2, 0:S])
                nc.gpsimd.partition_broadcast(q2rs_b[:], rqs[0:1, S : 2 * S])

                with tc.tile_pool(name="psB", bufs=1, space="PSUM") as psB:

                    def rope(dst, nrow, rmat, rs_b):
                        for soff, slen in S_BLOCKS:
                            pt = psB.tile([128, 512], F32, name="rot", tag="rot")
                            nc.tensor.matmul(
                                pt[:nrow, :slen],
                                rmat[:],
                                dst[:, soff : soff + slen],
                            )
                            nc.vector.tensor_mul(
                                tsin[:nrow, soff : soff + slen],
                                pt[:nrow, :slen],
                                sinb[:nrow, soff : soff + slen],
                            )
                        nc.vector.tensor_mul(
                            tcos[:nrow, :], dst[:], cosb[:nrow, :]
                        )
                        if rs_b is None:
                            nc.vector.tensor_add(
                                dst[:], tsin[:nrow, :], tcos[:nrow, :]
                            )
                        else:
                            nc.vector.tensor_add(
                                tsin[:nrow, :], tsin[:nrow, :], tcos[:nrow, :]
                            )
                            nc.vector.tensor_mul(
                                dst[:], tsin[:nrow, :], rs_b[:]
                            )

                    rope(qab, 128, rr, qrs_b)
                    rope(kab, 128, rr, None)
                    rope(q2t, HD, rr64, q2rs_b)
                    rope(k2t, HD, rr64, None)

                    # ---- attention ---------------------------------------
                    qsl = [qab[0:HD], qab[HD:128], q2t[:]]
                    ksl = [kab[0:HD], kab[HD:128], k2t[:]]

                    with (
                        tc.tile_pool(name="psS", bufs=2, space="PSUM") as psS,
                        tc.tile_pool(name="psAV", bufs=1, space="PSUM") as psAV,
                    ):
                        gidx = 0
                        for hf in range(2):
                            for h in range(HPC):
                                hoff = HALF * hf
                                av = psAV.tile(
                                    [65, HALF], F32, name="av", tag="av"
                                )
                                for j, (toff, tlen) in enumerate(T_TILES):
                                    pb = ppr.tile(
                                        [128, HALF], BF16, name="pb", tag="pb"
                                    )
                                    sc = psS.tile(
                                        [128, HALF], F32, name="sc", tag="sc"
                                    )
                                    for aoff, alen in HSUBS:
                                        nc.tensor.matmul(
                                            sc[:tlen, aoff : aoff + alen],
                                            ksl[h][:, toff : toff + tlen],
                                            qsl[h][
                                                :, hoff + aoff : hoff + aoff + alen
                                            ],
                                        )
                                    scl = (
                                        rk01[:tlen, 2 * j + h : 2 * j + h + 1]
                                        if h < 2
                                        else rk2[:tlen, j : j + 1]
                                    )
                                    nc.scalar.activation(
                                        pb[:tlen, :], sc[:tlen, :], AF.Exp,
                                        scale=scl,
                                    )
                                    for aoff, alen in HSUBS:
                                        nc.tensor.matmul(
                                            av[:, aoff : aoff + alen],
                                            vx[j][
                                                :tlen,
                                                (HD + 1) * h : (HD + 1) * (h + 1),
                                            ],
                                            pb[:tlen, aoff : aoff + alen],
                                            start=(j == 0),
                                            stop=(j == 13),
                                        )
                                # normalize: recip of denominator row, GPSIMD
                                # broadcast, single fused evac multiply
                                nc.vector.reciprocal(
                                    dsb[64:65, :], av[64:65, :]
                                )
                                for aoff, alen in HSUBS:
                                    nc.vector.tensor_copy(
                                        att3[h][
                                            :, hoff + aoff : hoff + aoff + alen
                                        ],
                                        av[0:HD, aoff : aoff + alen],
                                    )
                                for aoff, alen in HSUBS:
                                    po = psB.tile(
                                        [128, 512], F32, name="pod", tag="rb"
                                    )
                                    nc.tensor.matmul(
                                        po[0:HD, :alen],
                                        e1b[64:65, :],
                                        dsb[64:65, aoff : aoff + alen],
                                    )
                                    nc.vector.tensor_mul(
                                        att3[h][
                                            :, hoff + aoff : hoff + aoff + alen
                                        ],
                                        att3[h][
                                            :, hoff + aoff : hoff + aoff + alen
                                        ],
                                        po[0:HD, :alen],
                                    )
                                if gidx == 0:
                                    rope_half(k2t, HD, rr64, None, 0)
                                elif gidx == 1:
                                    rope_half(
                                        q2t, HD, rr64, (e2[0:1, 0:HD], 1, S), 0
                                    )
                                elif gidx == 2:
                                    rope_half(k2t, HD, rr64, None, 1)
                                elif gidx == 3:
                                    rope_half(
                                        q2t, HD, rr64, (e2[0:1, 0:HD], 1, S), 1
                                    )
                                gidx += 1

                    # ---- ffx/gate fused blocks + output blocks, both on
                    # the psA1 slot (PE filler during the ACT-bound attention
                    # window; output halves start as soon as the matching
                    # attention halves and ff columns are done) -------------
                    def filler_block(soff, slen):
                        for o in range(4, 10):
                            pt = psA1.tile([128, 512], F32, name="mmf", tag="mm")
                            acc = pt[:, :slen]
                            for c in range(6):
                                nc.tensor.matmul(
                                    acc,
                                    wf[c][:, 128 * o : 128 * (o + 1)],
                                    xn[c][:, soff : soff + slen],
                                    start=(c == 0),
                                    stop=(c == 5),
                                )
                            if o < 7:
                                nc.vector.tensor_copy(
                                    ffa[o - 4][:, soff : soff + slen], acc
                                )
                            else:
                                gs = pscr.tile(
                                    [128, 512], BF16, name="gs", tag="gs"
                                )
                                nc.scalar.activation(
                                    gs[:, :slen], acc, AF.Silu
                                )
                                nc.vector.tensor_mul(
                                    ffa[o - 7][:, soff : soff + slen],
                                    ffa[o - 7][:, soff : soff + slen],
                                    gs[:, :slen],
                                )

                    def out_block(soff, slen, pools):
                        for o in range(6):
                            pool = pools[o % len(pools)]
                            pt = pool.tile([128, 512], F32, name="oc", tag="oc")
                            acc = pt[:, :slen]
                            for h in range(3):
                                nc.tensor.matmul(
                                    acc,
                                    wam[
                                        :,
                                        HID * h + 128 * o : HID * h + 128 * (o + 1),
                                    ],
                                    att3[h][:, soff : soff + slen],
                                    start=(h == 0),
                                    stop=False,
                                )
                            for c in range(3):
                                nc.tensor.matmul(
                                    acc,
                                    wffm[
                                        :,
                                        HID * c + 128 * o : HID * c + 128 * (o + 1),
                                    ],
                                    ffa[c][:, soff : soff + slen],
                                    start=False,
                                    stop=(c == 2),
                                )
                            ob = pout.tile(
                                [128, 512], F32, name="obt", tag="obt"
                            )
                            nc.vector.scalar_tensor_tensor(
                                ob[:, :slen],
                                xn[o][:, soff : soff + slen],
                                risc[:, o : o + 1],
                                acc,
                                ALU.mult,
                                ALU.add,
                            )
                            nc.sync.dma_start(
                                outT[128 * o : 128 * (o + 1), soff : soff + slen],
                                ob[:, :slen],
                            )

                    filler_block(*S_BLOCKS[0])
                    filler_block(*S_BLOCKS[1])
                    filler_block(*S_BLOCKS[2])
                    filler_block(*S_BLOCKS[3])
                    psB.release()
                    psC = tc.alloc_tile_pool(name="psC", bufs=4, space="PSUM")
                    for soff, slen in HALVES[0] + HALVES[1]:
                        out_block(soff, slen, [psC])
                    psC.release()
                    psA1.release()
    _split_excess_waits(nc)
    return nc


# ---------------------------------------------------------------------------
# host-side preparation
# ---------------------------------------------------------------------------


def _axial_freqs():
    base = np.linspace(1.0, MAX_FREQ / 2, 8) * math.pi

    def ax(n):
        pos = np.linspace(-1.0, 1.0, n)
        return np.repeat(pos[:, None] * base[None, :], 2, axis=-1)

    fH = np.broadcast_to(ax(H)[:, None, None, :], (H, W, D, 16))
    fW = np.broadcast_to(ax(W)[None, :, None, :], (H, W, D, 16))
    fD = np.broadcast_to(ax(D)[None, None, :, :], (H, W, D, 16))
    return np.concatenate((fH, fW, fD), axis=-1).reshape(S, ROT)


def _bf16(a):
    import ml_dtypes

    return np.ascontiguousarray(np.asarray(a, np.float32)).astype(
        ml_dtypes.bfloat16
    )


def _prep_core_inputs(x, norm1_w, w_fused, b_fused, q_gamma, q_beta, k_gamma,
                      k_beta, w_attn, w_ff, b_ff):
    """Returns list of 8 in_maps (core = b*4 + r)."""
    f64 = np.float64
    w_fused = np.asarray(w_fused, f64)
    q_gamma = np.asarray(q_gamma, f64)
    k_gamma = np.asarray(k_gamma, f64)

    if np.any(np.asarray(b_fused)) or np.any(np.asarray(b_ff)):
        raise NotImplementedError("nonzero biases not supported by this kernel")
    if np.any(np.asarray(q_beta)) or np.any(np.asarray(k_beta)):
        raise NotImplementedError("nonzero q/k beta not supported by this kernel")
    if np.any(q_gamma == 0) or np.any(k_gamma == 0):
        raise NotImplementedError("zero gamma not supported by this kernel")

    M = np.eye(HD) - np.ones((HD, HD)) / HD
    Aq = np.diag(q_gamma) @ M
    Ak = np.diag(k_gamma) @ M
    R = np.zeros((HD, HD))
    for i in range(ROT // 2):
        R[2 * i, 2 * i + 1] = -1.0
        R[2 * i + 1, 2 * i] = 1.0
    R2 = np.zeros((128, 128))
    R2[0:64, 0:64] = R
    R2[64:128, 64:128] = R

    freqs = _axial_freqs()
    cos64 = np.ones((HD, S))
    sin64 = np.zeros((HD, S))
    cos64[:ROT, :] = np.cos(freqs).T
    sin64[:ROT, :] = np.sin(freqs).T
    cosT = _bf16(np.vstack([cos64, cos64]))
    sinT = _bf16(np.vstack([sin64, sin64]))

    wq_full = w_fused[MLP : MLP + HID]
    wk_full = w_fused[MLP + HID : MLP + 2 * HID]
    wv_full = w_fused[MLP + 2 * HID :]
    ffx_full = w_fused[: MLP // 2]
    gate_full = w_fused[MLP // 2 : MLP]

    nw = np.asarray(norm1_w, np.float32).reshape(6, 128).T
    wqk01 = np.zeros((128, 4))
    wqk01[0:64, 0] = 1.0 / (HD * q_gamma**2)
    wqk01[64:128, 1] = 1.0 / (HD * q_gamma**2)
    wqk01[0:64, 2] = 1.0 / k_gamma**2
    wqk01[64:128, 3] = 1.0 / k_gamma**2
    wqk2 = np.zeros((HD, 2))
    wqk2[:, 0] = 1.0 / (HD * q_gamma**2)
    wqk2[:, 1] = 1.0 / k_gamma**2
    rrm = np.zeros((128, 192))
    rrm[:, 0:128] = R2.T
    rrm[0:64, 128:192] = R.T
    eb2_np = np.zeros((2, 128))
    eb2_np[0, 0:64] = 1.0
    eb2_np[1, 64:128] = 1.0

    pad = np.zeros((64, HID))
    in_maps = []
    for core in range(N_CORES):
        b, r = divmod(core, TP)
        hs = [HPC * r + i for i in range(HPC)]
        q3 = [Aq @ wq_full[HD * h : HD * (h + 1)] for h in hs]
        k3 = [Ak @ wk_full[HD * h : HD * (h + 1)] for h in hs]
        ffx = ffx_full[FFPC * r : FFPC * (r + 1)]
        gate = gate_full[FFPC * r : FFPC * (r + 1)]
        wfT_np = np.vstack(
            [q3[0], q3[1], k3[0], k3[1], q3[2], pad, k3[2], pad, ffx, gate]
        ).T
        wv_mat = np.zeros((VCOLS, HID))
        for i, h in enumerate(hs):
            wv_mat[(HD + 1) * i : (HD + 1) * i + HD] = wv_full[HD * h : HD * (h + 1)]
        # [768, VCOLS] -> [128, 6*VCOLS] (chunk-c columns side by side)
        wvT_np = (
            wv_mat.T.reshape(6, 128, VCOLS)
            .transpose(1, 0, 2)
            .reshape(128, 6 * VCOLS)
        )
        acols = np.concatenate([np.arange(HD * h, HD * (h + 1)) for h in hs])
        waT_np = (
            np.asarray(w_attn, f64)[:, acols]
            .T.reshape(3, HD, HID)
            .transpose(1, 0, 2)
            .reshape(HD, 3 * HID)
        )
        wffT_np = (
            np.asarray(w_ff, f64)[:, FFPC * r : FFPC * (r + 1)]
            .T.reshape(3, 128, HID)
            .transpose(1, 0, 2)
            .reshape(128, 3 * HID)
        )
        nwrm = np.zeros((128, 12), np.float32)
        nwrm[:, 0:6] = nw
        nwrm[:, 6:12] = 1.0 if r == 0 else 0.0
        in_maps.append(
            {
                "xT": np.ascontiguousarray(
                    np.asarray(x[b], np.float32).reshape(HID, S)
                ),
                "wfT": _bf16(wfT_np),
                "wvT": _bf16(wvT_np),
                "waT": _bf16(waT_np),
                "wffT": _bf16(wffT_np),
                "cosT": cosT,
                "sinT": sinT,
                "rrT": _bf16(rrm),
                "nwrm": nwrm,
                "wqk01": _bf16(wqk01),
                "wqk2": _bf16(wqk2),
                "eb2": _bf16(eb2_np),
            }
        )
    return in_maps


_NC_CACHE = {}


def get_program():
    if "nc" not in _NC_CACHE:
        _NC_CACHE["nc"] = build_program()
    return _NC_CACHE["nc"]


def kernel(**inputs) -> np.ndarray:
    nc = get_program()
    in_maps = _prep_core_inputs(**inputs)
    res = bass_utils.run_bass_kernel_spmd(nc, in_maps, core_ids=list(range(N_CORES)))
    out = np.zeros((B, HID, H, W, D), np.float32)
    for core in range(N_CORES):
        b = core // TP
        out[b] += res.results[core]["outT"].reshape(HID, H, W, D)
    return out


# revision 59
# speedup vs baseline: 1.0938x; 1.0209x over previous
"""Trainium2 Bass kernel for nn_FullAttention_17789754540074.

Self-contained: takes the FULL inputs of reference.setup_inputs(), returns the
FULL output. Internally shards across 8 NeuronCores as 2-way data parallel
(batch) x 4-way tensor parallel (3 heads + 384 FF pairs per rank), runs one
SPMD Bass/Tile program via run_bass_kernel_spmd, and sums the 4 partial
outputs per batch on the host (the unshard step for partial-sum TP sharding).

Design notes (vs the earlier fp32r version; 265.5us -> 185.0us sim time):
  - bf16 activations/weights everywhere off the critical-precision path; x,
    PSUM accumulation, RMS stats, rk/rq rstds and the residual merge stay
    fp32 (hw rel err 1.8e-3).
  - per-channel-chunk RMS stats + interleaved x/wf chunk DMAs; aux tables
    ride the ACT DGE queue so the SP queue streams x/wf back-to-back.
  - single flat SBUF scope with emission order = scheduler priority:
    q01/k01 blocks -> their stats -> half-granular ropes -> q2/k2 -> v ->
    attention (hf-major) -> ffx/gate blocks -> output blocks.  The ffx/gate
    matmuls act as PE filler inside the ACT-exp-bound attention window.
  - stat PSUM tiles live in the roomy psA2 pool, NOT the rope-rotation
    slot: a stat tile's lifetime is bound to the serial ACT Ln/Exp wall and
    would block every later-emitted rope use of a shared slot (-8us).
  - k-side rstd Ln/Exp pairs consolidated to 2 ops via column-packed PSUM
    tiles; softmax denominator handled per-(head,half): DVE reciprocal of
    the ones-row of the AV accumulator, PE ones-column outer product to
    broadcast it, normalization fused into the PSUM evacuation multiply.
  - PSUM bank choreography (8 banks): psA2(6: q01/k01+kstats) and psB(1:
    stats/rot/denom-po) release into psS(4 scores)+psAV(2)+psA1(1 filler)
    during attention; output blocks run after on 2 freed banks, split into
    864-column halves so the first half starts right as hf0 heads finish.
  - NOTE: gpsimd/InstISA ops (partition_broadcast etc.) fail codegen on this
    walrus build ("ISA wrong length"); single-partition APs must start at a
    32-aligned partition; ACT Rsqrt/Reciprocal are blocked by bass.
"""

import math

import numpy as np

import concourse.bass as bass
import concourse.mybir as mybir
import concourse.tile as tile
from concourse import bass_utils
from concourse.vector_clock import ScopedClock

F32 = mybir.dt.float32
BF16 = mybir.dt.bfloat16
AF = mybir.ActivationFunctionType
ALU = mybir.AluOpType

HID, HEADS, HD, MLP = 768, 12, 64, 3072
B, H, W, D = 2, 12, 12, 12
S = H * W * D  # 1728
ROT = 48
MAX_FREQ = 256.0
EPS_GN, EPS_LN = 1e-6, 1e-5

N_CORES = 8
TP = 4
HPC = 3  # heads per core
FFPC = 384  # ff pairs per core
# fused rows: [q0 q1 | k0 k1 | q2 pad64 | k2 pad64 | ffx(384) | gate(384)]
NFUSED = 4 * 128 + 2 * FFPC  # 1280
VCOLS = HPC * (HD + 1)  # 195: per head [v(64), one]

S_BLOCKS = [(0, 512), (512, 512), (1024, 448), (1472, 256)]
T_TILES = [(128 * j, 128) for j in range(13)] + [(1664, 64)]
HALF = S // 2  # 864
HSUBS = [(0, 512), (512, 352)]


class TileContextSplitDrain(tile.TileContext):
    """TileContext whose kernel-tail drain splits its semaphore waits across
    single-wait sync NOPs — the walrus build here rejects >2 sync waits on one
    SP CTRL instruction ("Too many sync wait commands")."""

    def _drain_and_barrier(self, tick_clock, wait_clock):
        probe = self.nc.sync.nop(nofuse=True)
        wait_clock.add_sem_waits(
            probe.ins, ScopedClock({None: tick_clock.global_clock})
        )
        si = probe.ins.sync_info
        waits = list(si.on_wait) if si is not None else []
        if si is not None:
            si.on_wait = waits[:1]
        for w in waits[1:]:
            n = self.nc.sync.nop(nofuse=True)
            nsi = n.ins.sync_info
            if nsi is None:
                n.ins.sync_info = mybir.SyncInfo(on_wait=[w], on_update=[])
            else:
                nsi.on_wait.append(w)
        self.nc.sync.drain()
        self.nc.all_engine_barrier()
        popped = self.nc._tile_sem_poison_stack.pop()
        assert popped is self._sem_poison
        self.nc.clear_and_free_semaphores(list(self.sems.allocated().values()))
        self.nc.all_engine_barrier()


def _split_excess_waits(nc, maxw=1):
    """walrus in this container caps sync waits per instruction; move extras
    onto preceding same-engine NOPs (waits execute in program order)."""
    nid = 0
    for bb in nc.m.functions[0].blocks:
        insts = bb.instructions
        i = 0
        while i < len(insts):
            inst = insts[i]
            si = inst.sync_info
            nw = len(si.on_wait) if si is not None and si.on_wait else 0
            if nw > maxw:
                waits = list(si.on_wait)
                si.on_wait = waits[-maxw:]
                extra = waits[:-maxw]
                pos = i
                for k in range(0, len(extra), maxw):
                    nop = mybir.InstNoOp(
                        name=f"I-waitsplit-{nid}", ins=[], outs=[]
                    )
                    nop.engine = inst.engine
                    nop.sync_info = mybir.SyncInfo(
                        on_wait=extra[k : k + maxw], on_update=[]
                    )
                    insts.insert(pos, nop)
                    nc.register_instruction(nop)
                    pos += 1
                    i += 1
                    nid += 1
            i += 1


def build_program():
    nc = bass.Bass(trn_type="TRN2")

    xT = nc.dram_tensor("xT", [HID, S], F32, kind="ExternalInput")
    wfT = nc.dram_tensor("wfT", [HID, NFUSED], BF16, kind="ExternalInput")
    # wv / wa / wff pre-flattened on host to single-DMA layouts
    wvT = nc.dram_tensor("wvT", [128, 6 * VCOLS], BF16, kind="ExternalInput")
    waT = nc.dram_tensor("waT", [HD, HPC * HID], BF16, kind="ExternalInput")
    wffT = nc.dram_tensor("wffT", [128, 3 * HID], BF16, kind="ExternalInput")
    cosT = nc.dram_tensor("cosT", [128, S], BF16, kind="ExternalInput")
    sinT = nc.dram_tensor("sinT", [128, S], BF16, kind="ExternalInput")
    # rr [128,128] with rr64 packed at rows 0:64, cols 128:192
    rrT = nc.dram_tensor("rrT", [128, 192], BF16, kind="ExternalInput")
    # nw cols 0:6, rmask cols 6:12
    nwrm = nc.dram_tensor("nwrm", [128, 12], F32, kind="ExternalInput")
    # wq01 cols 0:2, wk01 cols 2:4
    wqk01 = nc.dram_tensor("wqk01", [128, 4], BF16, kind="ExternalInput")
    # wq2 col 0, wk2 col 1
    wqk2 = nc.dram_tensor("wqk2", [HD, 2], BF16, kind="ExternalInput")
    eb2 = nc.dram_tensor("eb2", [2, 128], BF16, kind="ExternalInput")
    outT = nc.dram_tensor("outT", [HID, S], F32, kind="ExternalOutput")

    with TileContextSplitDrain(nc) as tc, nc.allow_low_precision(
        reason="bf16 activations; accumulation and stats stay fp32"
    ):
        with (
            tc.tile_pool(name="big", bufs=1) as pbg,
            tc.tile_pool(name="wts", bufs=1) as pwt,
            tc.tile_pool(name="scr", bufs=3) as pscr,
            tc.tile_pool(name="probs", bufs=16) as ppr,
            tc.tile_pool(name="outp", bufs=4) as pout,
            tc.tile_pool(name="small", bufs=1) as psm,
        ):
            # ---- persistent bf16 tiles -----------------------------------
            xn = [pbg.tile([128, S], BF16, name=f"xn{c}", tag=f"xn{c}")
                  for c in range(6)]
            ffa = [pbg.tile([128, S], BF16, name=f"ffa{i}", tag=f"ffa{i}")
                   for i in range(3)]
            vx = [pbg.tile([128, VCOLS], BF16, name=f"vx{j}", tag=f"vx{j}")
                  for j in range(14)]
            qab = pbg.tile([128, S], BF16, name="qab", tag="qab")
            kab = pbg.tile([128, S], BF16, name="kab", tag="kab")
            q2t = pbg.tile([HD, S], BF16, name="q2t", tag="q2t")
            k2t = pbg.tile([HD, S], BF16, name="k2t", tag="k2t")
            cosb = pbg.tile([128, S], BF16, name="cosb", tag="cosb")
            sinb = pbg.tile([128, S], BF16, name="sinb", tag="sinb")
            tsin = pbg.tile([128, S], BF16, name="tsin", tag="tsin")
            tcos = pbg.tile([128, S], BF16, name="tcos", tag="tcos")
            sqA = pbg.tile([128, S], BF16, name="sqA", tag="sqA")
            sqB = pbg.tile([HD, S], BF16, name="sqB", tag="sqB")
            att3 = [pbg.tile([HD, S], BF16, name=f"att{h}", tag=f"att{h}")
                    for h in range(3)]
            rqs = pbg.tile([2, 2 * S], BF16, name="rqs", tag="rqs")
            e2 = pbg.tile([2, 128], BF16, name="e2", tag="e2")
            e1b = pbg.tile([65, HD], BF16, name="e1b", tag="e1b")
            dsb = pbg.tile([65, HALF], BF16, name="dsb", tag="dsb")
            sqsc = pbg.tile([128, S], BF16, name="sqsc", tag="sqsc")

            rrm = pwt.tile([128, 192], BF16, name="rrm", tag="rrm")
            rr = rrm[:, 0:128]
            rr64 = rrm[0:HD, 128:192]
            wf = [pwt.tile([128, NFUSED], BF16, name=f"wf{c}", tag=f"wf{c}")
                  for c in range(6)]
            wvm = pwt.tile([128, 6 * VCOLS], BF16, name="wvm", tag="wvm")
            wam = pwt.tile([HD, HPC * HID], BF16, name="wam", tag="wam")
            wffm = pwt.tile([128, 3 * HID], BF16, name="wffm", tag="wffm")
            wqk01t = pwt.tile([128, 4], BF16, name="wqk01t", tag="wqk01t")
            wq01t = wqk01t[:, 0:2]
            wk01t = wqk01t[:, 2:4]
            wqk2t = pwt.tile([HD, 2], BF16, name="wqk2t", tag="wqk2t")
            wq2t = wqk2t[:, 0:1]
            wk2t = wqk2t[:, 1:2]

            nwrmt = psm.tile([128, 12], F32, name="nwrmt", tag="nwrmt")
            nwt = nwrmt[:, 0:6]
            rmk = nwrmt[:, 6:12]
            ss6 = psm.tile([128, 6], F32, name="ss6", tag="ss6")
            scale6 = psm.tile([128, 6], F32, name="scale6", tag="scale6")
            risc0 = psm.tile([128, 6], F32, name="risc0", tag="risc0")
            risc = psm.tile([128, 6], F32, name="risc", tag="risc")
            rk01 = psm.tile([128, 28], F32, name="rk01", tag="rk01")
            rk2 = psm.tile([128, 14], F32, name="rk2", tag="rk2")
            lnq = psm.tile([2, 512], F32, name="lnq", tag="lnq")
            cgn = psm.tile([128, 1], F32, name="cgn", tag="cgn")
            cln2 = psm.tile([2, 1], F32, name="cln2", tag="cln2")
            cln1 = psm.tile([1, 1], F32, name="cln1", tag="cln1")
            cl64 = psm.tile([128, 1], F32, name="cl64", tag="cl64")

            nc.vector.memset(e1b[64:65, :], 1.0)
            nc.vector.memset(cgn[:], EPS_GN)
            nc.vector.memset(cln2[:], EPS_LN)
            nc.vector.memset(cln1[:], EPS_LN)
            nc.vector.memset(cl64[:], 64.0 * EPS_LN)

            # tiny tables on the scalar DGE queue (keeps the SP queue free
            # for the x/wf stream); nwrm first — needed by chunk-0 stats
            nc.scalar.dma_start(nwrmt[:], nwrm[:])
            nc.scalar.dma_start(wqk01t[:], wqk01[:])
            nc.scalar.dma_start(wqk2t[:], wqk2[:])
            nc.scalar.dma_start(e2[:], eb2[:])
            nc.scalar.dma_start(rrm[:], rrT[:])

            with tc.tile_pool(name="xr", bufs=1) as pxr:
                xraw = [pxr.tile([128, S], F32, name=f"xr{c}", tag=f"xr{c}")
                        for c in range(6)]

                # interleaved x/wf chunk stream: the double-buffered q01/k01
                # PSUM groups pre-accumulate c-chunks as they land
                for c in range(6):
                    nc.sync.dma_start(xraw[c][:], xT[128 * c : 128 * (c + 1), :])
                    nc.sync.dma_start(wf[c][:], wfT[128 * c : 128 * (c + 1), :])
                nc.sync.dma_start(cosb[:], cosT[:])
                nc.sync.dma_start(sinb[:], sinT[:])
                nc.sync.dma_start(wvm[:], wvT[:])
                nc.sync.dma_start(wam[:], waT[:])
                nc.sync.dma_start(wffm[:], wffT[:])

                # ---- per-chunk RMSGroupNorm stats + normalized x ---------
                for c in range(6):
                    nc.scalar.activation(
                        sqsc[:],
                        xraw[c][:],
                        AF.Square,
                        accum_out=ss6[:, c : c + 1],
                    )
                    # std = sqrt(ss/S + eps); scale6 = nw/std; risc = std/nw
                    nc.scalar.activation(
                        ss6[:, c : c + 1], ss6[:, c : c + 1], AF.Sqrt,
                        bias=cgn[:], scale=1.0 / S,
                    )
                    nc.vector.reciprocal(
                        risc0[:, c : c + 1], ss6[:, c : c + 1]
                    )
                    nc.vector.tensor_mul(
                        scale6[:, c : c + 1], risc0[:, c : c + 1],
                        nwt[:, c : c + 1],
                    )
                    nc.vector.reciprocal(
                        risc0[:, c : c + 1], scale6[:, c : c + 1]
                    )
                    nc.vector.tensor_mul(
                        risc[:, c : c + 1], risc0[:, c : c + 1], rmk[:, c : c + 1]
                    )
                    nc.vector.tensor_scalar(
                        xn[c][:], xraw[c][:], scale6[:, c : c + 1], None, ALU.mult
                    )

            # ---- fused projection: q01, k01, q2, k2 ----------------------
            qk_dst = [qab, kab, q2t, k2t]
            with tc.tile_pool(name="psA2", bufs=5, space="PSUM") as psA2:
                for o in range(4):
                    for soff, slen in S_BLOCKS:
                        pt = psA2.tile([128, 512], F32, name="mm", tag="mm")
                        acc = pt[:, :slen]
                        for c in range(6):
                            nc.tensor.matmul(
                                acc,
                                wf[c][:, 128 * o : 128 * (o + 1)],
                                xn[c][:, soff : soff + slen],
                                start=(c == 0),
                                stop=(c == 5),
                            )
                        if o < 2:
                            if o == 0:
                                nc.vector.tensor_copy(
                                    qk_dst[o][:, soff : soff + slen], acc
                                )
                            else:
                                nc.scalar.activation(
                                    qk_dst[o][:, soff : soff + slen], acc,
                                    AF.Copy,
                                )
                        else:
                            nc.scalar.activation(
                                qk_dst[o][:, soff : soff + slen], acc[0:HD, :],
                                AF.Copy,
                            )

            with tc.tile_pool(name="psA1", bufs=1, space="PSUM") as psA1:
                # ---- v projection (token-major) --------------------------
                for j, (toff, tlen) in enumerate(T_TILES):
                    pt = psA1.tile([128, 512], F32, name="mmv", tag="mm")
                    acc = pt[:tlen, :VCOLS]
                    for c in range(6):
                        nc.tensor.matmul(
                            acc,
                            xn[c][:, toff : toff + tlen],
                            wv[c][:],
                            start=(c == 0),
                            stop=(c == 5),
                        )
                    nc.scalar.activation(vx[j][:tlen, :], acc, AF.Copy)
                    # ones columns for the softmax denominators
                    nc.vector.memset(vx[j][:tlen, HD : VCOLS : HD + 1], 1.0)

                # ---- q/k layernorm rstd stats ----------------------------
                with tc.tile_pool(name="psStat", bufs=1, space="PSUM") as psV:
                    # q side: rstd rows [2, S] (h0/h1) + [1, S] (h2);
                    # rsqrt as exp(-0.5 ln(var + eps))
                    nc.vector.tensor_mul(sqA[:], qab[:], qab[:])
                    for soff, slen in S_BLOCKS:
                        pt = psV.tile([2, 512], F32, name="vq", tag="vq")
                        nc.tensor.matmul(
                            pt[:, :slen], wq01t[:], sqA[:, soff : soff + slen]
                        )
                        nc.scalar.activation(
                            lnq[0:2, :slen], pt[:, :slen], AF.Ln, bias=cln2[:]
                        )
                        nc.scalar.activation(
                            rqs[0:2, soff : soff + slen], lnq[0:2, :slen],
                            AF.Exp, scale=-0.5,
                        )
                    nc.vector.tensor_mul(sqB[:], q2t[:], q2t[:])
                    for soff, slen in S_BLOCKS:
                        pt = psV.tile([2, 512], F32, name="vq2", tag="vq")
                        nc.tensor.matmul(
                            pt[0:1, :slen], wq2t[:], sqB[:, soff : soff + slen]
                        )
                        nc.scalar.activation(
                            lnq[0:1, :slen], pt[0:1, :slen], AF.Ln, bias=cln1[:]
                        )
                        nc.scalar.activation(
                            rqs[0:1, S + soff : S + soff + slen],
                            lnq[0:1, :slen],
                            AF.Exp, scale=-0.5,
                        )

                    # k side: rstd/8 columns, consolidated Rsqrts
                    nc.vector.tensor_mul(sqA[:], kab[:], kab[:])
                    nc.vector.tensor_mul(sqB[:], k2t[:], k2t[:])
                    vst = psV.tile([128, 48], F32, name="vst", tag="vst")
                    nc.vector.memset(vst[:], 1.0)
                    for j, (toff, tlen) in enumerate(T_TILES):
                        nc.tensor.matmul(
                            vst[:tlen, 2 * j : 2 * j + 2],
                            sqA[:, toff : toff + tlen],
                            wk01t[:],
                        )
                        nc.tensor.matmul(
                            vst[:tlen, 28 + j : 29 + j],
                            sqB[:, toff : toff + tlen],
                            wk2t[:],
                        )
                    nc.scalar.activation(
                        rk01[:], vst[:, 0:28], AF.Ln, bias=cl64[:]
                    )
                    nc.scalar.activation(
                        rk01[:], rk01[:], AF.Exp, scale=-0.5
                    )
                    nc.scalar.activation(
                        rk2[:], vst[:, 28:42], AF.Ln, bias=cl64[:]
                    )
                    nc.scalar.activation(
                        rk2[:], rk2[:], AF.Exp, scale=-0.5
                    )

                # rstd_q broadcasts on the idle GPSIMD engine
                nc.gpsimd.partition_broadcast(qrs_b[0:HD, :], rqs[0:1, 0:S])
                nc.gpsimd.partition_broadcast(qrs_b[HD:128, :], rqs[1:2, 0:S])
                nc.gpsimd.partition_broadcast(q2rs_b[:], rqs[0:1, S : 2 * S])

                with tc.tile_pool(name="psB", bufs=1, space="PSUM") as psB:

                    def rope(dst, nrow, rmat, rs_b):
                        for soff, slen in S_BLOCKS:
                            pt = psB.tile([128, 512], F32, name="rot", tag="rot")
                            nc.tensor.matmul(
                                pt[:nrow, :slen],
                                rmat[:],
                                dst[:, soff : soff + slen],
                            )
                            nc.vector.tensor_mul(
                                tsin[:nrow, soff : soff + slen],
                                pt[:nrow, :slen],
                                sinb[:nrow, soff : soff + slen],
                            )
                        nc.vector.tensor_mul(
                            tcos[:nrow, :], dst[:], cosb[:nrow, :]
                        )
                        if rs_b is None:
                            nc.vector.tensor_add(
                                dst[:], tsin[:nrow, :], tcos[:nrow, :]
                            )
                        else:
                            nc.vector.tensor_add(
                                tsin[:nrow, :], tsin[:nrow, :], tcos[:nrow, :]
                            )
                            nc.vector.tensor_mul(
                                dst[:], tsin[:nrow, :], rs_b[:]
                            )

                    rope(qab, 128, rr, qrs_b)
                    rope(kab, 128, rr, None)
                    rope(q2t, HD, rr64, q2rs_b)
                    rope(k2t, HD, rr64, None)

                    # ---- attention ---------------------------------------
                    qsl = [qab[0:HD], qab[HD:128], q2t[:]]
                    ksl = [kab[0:HD], kab[HD:128], k2t[:]]

                    with (
                        tc.tile_pool(name="psS", bufs=2, space="PSUM") as psS,
                        tc.tile_pool(name="psAV", bufs=1, space="PSUM") as psAV,
                    ):
                        for hf in range(2):
                            for h in range(HPC):
                                hoff = HALF * hf
                                av = psAV.tile(
                                    [65, HALF], F32, name="av", tag="av"
                                )
                                for j, (toff, tlen) in enumerate(T_TILES):
                                    pb = ppr.tile(
                                        [128, HALF], BF16, name="pb", tag="pb"
                                    )
                                    sc = psS.tile(
                                        [128, HALF], F32, name="sc", tag="sc"
                                    )
                                    for aoff, alen in HSUBS:
                                        nc.tensor.matmul(
                                            sc[:tlen, aoff : aoff + alen],
                                            ksl[h][:, toff : toff + tlen],
                                            qsl[h][
                                                :, hoff + aoff : hoff + aoff + alen
                                            ],
                                        )
                                    scl = (
                                        rk01[:tlen, 2 * j + h : 2 * j + h + 1]
                                        if h < 2
                                        else rk2[:tlen, j : j + 1]
                                    )
                                    nc.scalar.activation(
                                        pb[:tlen, :], sc[:tlen, :], AF.Exp,
                                        scale=scl,
                                    )
                                    for aoff, alen in HSUBS:
                                        nc.tensor.matmul(
                                            av[:, aoff : aoff + alen],
                                            vx[j][
                                                :tlen,
                                                (HD + 1) * h : (HD + 1) * (h + 1),
                                            ],
                                            pb[:tlen, aoff : aoff + alen],
                                            start=(j == 0),
                                            stop=(j == 13),
                                        )
                                # normalize: recip of denominator row, GPSIMD
                                # broadcast, single fused evac multiply
                                nc.vector.reciprocal(
                                    dsb[64:65, :], av[64:65, :]
                                )
                                for aoff, alen in HSUBS:
                                    nc.vector.tensor_copy(
                                        att3[h][
                                            :, hoff + aoff : hoff + aoff + alen
                                        ],
                                        av[0:HD, aoff : aoff + alen],
                                    )
                                for aoff, alen in HSUBS:
                                    po = psB.tile(
                                        [128, 512], F32, name="pod", tag="rb"
                                    )
                                    nc.tensor.matmul(
                                        po[0:HD, :alen],
                                        e1b[64:65, :],
                                        dsb[64:65, aoff : aoff + alen],
                                    )
                                    nc.vector.tensor_mul(
                                        att3[h][
                                            :, hoff + aoff : hoff + aoff + alen
                                        ],
                                        att3[h][
                                            :, hoff + aoff : hoff + aoff + alen
                                        ],
                                        po[0:HD, :alen],
                                    )

                    # ---- ffx/gate fused blocks + output blocks, both on
                    # the psA1 slot (PE filler during the ACT-bound attention
                    # window; output halves start as soon as the matching
                    # attention halves and ff columns are done) -------------
                    def filler_block(soff, slen):
                        for o in range(4, 10):
                            pt = psA1.tile([128, 512], F32, name="mmf", tag="mm")
                            acc = pt[:, :slen]
                            for c in range(6):
                                nc.tensor.matmul(
                                    acc,
                                    wf[c][:, 128 * o : 128 * (o + 1)],
                                    xn[c][:, soff : soff + slen],
                                    start=(c == 0),
                                    stop=(c == 5),
                                )
                            if o < 7:
                                nc.vector.tensor_copy(
                                    ffa[o - 4][:, soff : soff + slen], acc
                                )
                            else:
                                gs = pscr.tile(
                                    [128, 512], BF16, name="gs", tag="gs"
                                )
                                nc.scalar.activation(
                                    gs[:, :slen], acc, AF.Silu
                                )
                                nc.vector.tensor_mul(
                                    ffa[o - 7][:, soff : soff + slen],
                                    ffa[o - 7][:, soff : soff + slen],
                                    gs[:, :slen],
                                )

                    def out_block(soff, slen, pools):
                        for o in range(6):
                            pool = pools[o % len(pools)]
                            pt = pool.tile([128, 512], F32, name="oc", tag="oc")
                            acc = pt[:, :slen]
                            for h in range(3):
                                nc.tensor.matmul(
                                    acc,
                                    wam[
                                        :,
                                        HID * h + 128 * o : HID * h + 128 * (o + 1),
                                    ],
                                    att3[h][:, soff : soff + slen],
                                    start=(h == 0),
                                    stop=False,
                                )
                            for c in range(3):
                                nc.tensor.matmul(
                                    acc,
                                    wffm[
                                        :,
                                        HID * c + 128 * o : HID * c + 128 * (o + 1),
                                    ],
                                    ffa[c][:, soff : soff + slen],
                                    start=False,
                                    stop=(c == 2),
                                )
                            ob = pout.tile(
                                [128, 512], F32, name="obt", tag="obt"
                            )
                            nc.vector.scalar_tensor_tensor(
                                ob[:, :slen],
                                xn[o][:, soff : soff + slen],
                                risc[:, o : o + 1],
                                acc,
                                ALU.mult,
                                ALU.add,
                            )
                            nc.sync.dma_start(
                                outT[128 * o : 128 * (o + 1), soff : soff + slen],
                                ob[:, :slen],
                            )

                    filler_block(*S_BLOCKS[0])
                    filler_block(*S_BLOCKS[1])
                    filler_block(*S_BLOCKS[2])
                    filler_block(*S_BLOCKS[3])
                    psB.release()
                    psC = tc.alloc_tile_pool(name="psC", bufs=4, space="PSUM")
                    for soff, slen in HALVES[0] + HALVES[1]:
                        out_block(soff, slen, [psC])
                    psC.release()
                    psA1.release()
    _split_excess_waits(nc)
    return nc


# ---------------------------------------------------------------------------
# host-side preparation
# ---------------------------------------------------------------------------


def _axial_freqs():
    base = np.linspace(1.0, MAX_FREQ / 2, 8) * math.pi

    def ax(n):
        pos = np.linspace(-1.0, 1.0, n)
        return np.repeat(pos[:, None] * base[None, :], 2, axis=-1)

    fH = np.broadcast_to(ax(H)[:, None, None, :], (H, W, D, 16))
    fW = np.broadcast_to(ax(W)[None, :, None, :], (H, W, D, 16))
    fD = np.broadcast_to(ax(D)[None, None, :, :], (H, W, D, 16))
    return np.concatenate((fH, fW, fD), axis=-1).reshape(S, ROT)


def _bf16(a):
    import ml_dtypes

    return np.ascontiguousarray(np.asarray(a, np.float32)).astype(
        ml_dtypes.bfloat16
    )


def _prep_core_inputs(x, norm1_w, w_fused, b_fused, q_gamma, q_beta, k_gamma,
                      k_beta, w_attn, w_ff, b_ff):
    """Returns list of 8 in_maps (core = b*4 + r)."""
    f64 = np.float64
    w_fused = np.asarray(w_fused, f64)
    q_gamma = np.asarray(q_gamma, f64)
    k_gamma = np.asarray(k_gamma, f64)

    if np.any(np.asarray(b_fused)) or np.any(np.asarray(b_ff)):
        raise NotImplementedError("nonzero biases not supported by this kernel")
    if np.any(np.asarray(q_beta)) or np.any(np.asarray(k_beta)):
        raise NotImplementedError("nonzero q/k beta not supported by this kernel")
    if np.any(q_gamma == 0) or np.any(k_gamma == 0):
        raise NotImplementedError("zero gamma not supported by this kernel")

    M = np.eye(HD) - np.ones((HD, HD)) / HD
    Aq = np.diag(q_gamma) @ M
    Ak = np.diag(k_gamma) @ M
    R = np.zeros((HD, HD))
    for i in range(ROT // 2):
        R[2 * i, 2 * i + 1] = -1.0
        R[2 * i + 1, 2 * i] = 1.0
    R2 = np.zeros((128, 128))
    R2[0:64, 0:64] = R
    R2[64:128, 64:128] = R

    freqs = _axial_freqs()
    cos64 = np.ones((HD, S))
    sin64 = np.zeros((HD, S))
    cos64[:ROT, :] = np.cos(freqs).T
    sin64[:ROT, :] = np.sin(freqs).T
    cosT = _bf16(np.vstack([cos64, cos64]))
    sinT = _bf16(np.vstack([sin64, sin64]))

    wq_full = w_fused[MLP : MLP + HID]
    wk_full = w_fused[MLP + HID : MLP + 2 * HID]
    wv_full = w_fused[MLP + 2 * HID :]
    ffx_full = w_fused[: MLP // 2]
    gate_full = w_fused[MLP // 2 : MLP]

    nw = np.asarray(norm1_w, np.float32).reshape(6, 128).T
    wqk01 = np.zeros((128, 4))
    wqk01[0:64, 0] = 1.0 / (HD * q_gamma**2)
    wqk01[64:128, 1] = 1.0 / (HD * q_gamma**2)
    wqk01[0:64, 2] = 1.0 / k_gamma**2
    wqk01[64:128, 3] = 1.0 / k_gamma**2
    wqk2 = np.zeros((HD, 2))
    wqk2[:, 0] = 1.0 / (HD * q_gamma**2)
    wqk2[:, 1] = 1.0 / k_gamma**2
    rrm = np.zeros((128, 192))
    rrm[:, 0:128] = R2.T
    rrm[0:64, 128:192] = R.T
    eb2_np = np.zeros((2, 128))
    eb2_np[0, 0:64] = 1.0
    eb2_np[1, 64:128] = 1.0

    pad = np.zeros((64, HID))
    in_maps = []
    for core in range(N_CORES):
        b, r = divmod(core, TP)
        hs = [HPC * r + i for i in range(HPC)]
        q3 = [Aq @ wq_full[HD * h : HD * (h + 1)] for h in hs]
        k3 = [Ak @ wk_full[HD * h : HD * (h + 1)] for h in hs]
        ffx = ffx_full[FFPC * r : FFPC * (r + 1)]
        gate = gate_full[FFPC * r : FFPC * (r + 1)]
        wfT_np = np.vstack(
            [q3[0], q3[1], k3[0], k3[1], q3[2], pad, k3[2], pad, ffx, gate]
        ).T
        wv_mat = np.zeros((VCOLS, HID))
        for i, h in enumerate(hs):
            wv_mat[(HD + 1) * i : (HD + 1) * i + HD] = wv_full[HD * h : HD * (h + 1)]
        # [768, VCOLS] -> [128, 6*VCOLS] (chunk-c columns side by side)
        wvT_np = (
            wv_mat.T.reshape(6, 128, VCOLS)
            .transpose(1, 0, 2)
            .reshape(128, 6 * VCOLS)
        )
        acols = np.concatenate([np.arange(HD * h, HD * (h + 1)) for h in hs])
        waT_np = (
            np.asarray(w_attn, f64)[:, acols]
            .T.reshape(3, HD, HID)
            .transpose(1, 0, 2)
            .reshape(HD, 3 * HID)
        )
        wffT_np = (
            np.asarray(w_ff, f64)[:, FFPC * r : FFPC * (r + 1)]
            .T.reshape(3, 128, HID)
            .transpose(1, 0, 2)
            .reshape(128, 3 * HID)
        )
        nwrm = np.zeros((128, 12), np.float32)
        nwrm[:, 0:6] = nw
        nwrm[:, 6:12] = 1.0 if r == 0 else 0.0
        in_maps.append(
            {
                "xT": np.ascontiguousarray(
                    np.asarray(x[b], np.float32).reshape(HID, S)
                ),
                "wfT": _bf16(wfT_np),
                "wvT": _bf16(wvT_np),
                "waT": _bf16(waT_np),
                "wffT": _bf16(wffT_np),
                "cosT": cosT,
                "sinT": sinT,
                "rrT": _bf16(rrm),
                "nwrm": nwrm,
                "wqk01": _bf16(wqk01),
                "wqk2": _bf16(wqk2),
                "eb2": _bf16(eb2_np),
            }
        )
    return in_maps


_NC_CACHE = {}


def get_program():
    if "nc" not in _NC_CACHE:
        _NC_CACHE["nc"] = build_program()
    return _NC_CACHE["nc"]


def kernel(**inputs) -> np.ndarray:
    nc = get_program()
    in_maps = _prep_core_inputs(**inputs)
    res = bass_utils.run_bass_kernel_spmd(nc, in_maps, core_ids=list(range(N_CORES)))
    out = np.zeros((B, HID, H, W, D), np.float32)
    for core in range(N_CORES):
        b = core // TP
        out[b] += res.results[core]["outT"].reshape(HID, H, W, D)
    return out


# revision 60
# speedup vs baseline: 1.1034x; 1.0088x over previous
"""Trainium2 Bass kernel for nn_FullAttention_17789754540074.

Self-contained: takes the FULL inputs of reference.setup_inputs(), returns the
FULL output. Internally shards across 8 NeuronCores as 2-way data parallel
(batch) x 4-way tensor parallel (3 heads + 384 FF pairs per rank), runs one
SPMD Bass/Tile program via run_bass_kernel_spmd, and sums the 4 partial
outputs per batch on the host (the unshard step for partial-sum TP sharding).

Design notes (vs the earlier fp32r version; 265.5us -> 185.0us sim time):
  - bf16 activations/weights everywhere off the critical-precision path; x,
    PSUM accumulation, RMS stats, rk/rq rstds and the residual merge stay
    fp32 (hw rel err 1.8e-3).
  - per-channel-chunk RMS stats + interleaved x/wf chunk DMAs; aux tables
    ride the ACT DGE queue so the SP queue streams x/wf back-to-back.
  - single flat SBUF scope with emission order = scheduler priority:
    q01/k01 blocks -> their stats -> half-granular ropes -> q2/k2 -> v ->
    attention (hf-major) -> ffx/gate blocks -> output blocks.  The ffx/gate
    matmuls act as PE filler inside the ACT-exp-bound attention window.
  - stat PSUM tiles live in the roomy psA2 pool, NOT the rope-rotation
    slot: a stat tile's lifetime is bound to the serial ACT Ln/Exp wall and
    would block every later-emitted rope use of a shared slot (-8us).
  - k-side rstd Ln/Exp pairs consolidated to 2 ops via column-packed PSUM
    tiles; softmax denominator handled per-(head,half): DVE reciprocal of
    the ones-row of the AV accumulator, PE ones-column outer product to
    broadcast it, normalization fused into the PSUM evacuation multiply.
  - PSUM bank choreography (8 banks): psA2(6: q01/k01+kstats) and psB(1:
    stats/rot/denom-po) release into psS(4 scores)+psAV(2)+psA1(1 filler)
    during attention; output blocks run after on 2 freed banks, split into
    864-column halves so the first half starts right as hf0 heads finish.
  - NOTE: gpsimd/InstISA ops (partition_broadcast etc.) fail codegen on this
    walrus build ("ISA wrong length"); single-partition APs must start at a
    32-aligned partition; ACT Rsqrt/Reciprocal are blocked by bass.
"""

import math

import numpy as np

import concourse.bass as bass
import concourse.mybir as mybir
import concourse.tile as tile
from concourse import bass_utils
from concourse.vector_clock import ScopedClock

F32 = mybir.dt.float32
BF16 = mybir.dt.bfloat16
AF = mybir.ActivationFunctionType
ALU = mybir.AluOpType

HID, HEADS, HD, MLP = 768, 12, 64, 3072
B, H, W, D = 2, 12, 12, 12
S = H * W * D  # 1728
ROT = 48
MAX_FREQ = 256.0
EPS_GN, EPS_LN = 1e-6, 1e-5

N_CORES = 8
TP = 4
HPC = 3  # heads per core
FFPC = 384  # ff pairs per core
# fused rows: [q0 q1 | k0 k1 | q2 pad64 | k2 pad64 | ffx(384) | gate(384)]
NFUSED = 4 * 128 + 2 * FFPC  # 1280
VCOLS = HPC * (HD + 1)  # 195: per head [v(64), one]

S_BLOCKS = [(0, 512), (512, 512), (1024, 448), (1472, 256)]
T_TILES = [(128 * j, 128) for j in range(13)] + [(1664, 64)]
HALF = S // 2  # 864
HSUBS = [(0, 512), (512, 352)]


class TileContextSplitDrain(tile.TileContext):
    """TileContext whose kernel-tail drain splits its semaphore waits across
    single-wait sync NOPs — the walrus build here rejects >2 sync waits on one
    SP CTRL instruction ("Too many sync wait commands")."""

    def _drain_and_barrier(self, tick_clock, wait_clock):
        probe = self.nc.sync.nop(nofuse=True)
        wait_clock.add_sem_waits(
            probe.ins, ScopedClock({None: tick_clock.global_clock})
        )
        si = probe.ins.sync_info
        waits = list(si.on_wait) if si is not None else []
        if si is not None:
            si.on_wait = waits[:1]
        for w in waits[1:]:
            n = self.nc.sync.nop(nofuse=True)
            nsi = n.ins.sync_info
            if nsi is None:
                n.ins.sync_info = mybir.SyncInfo(on_wait=[w], on_update=[])
            else:
                nsi.on_wait.append(w)
        self.nc.sync.drain()
        self.nc.all_engine_barrier()
        popped = self.nc._tile_sem_poison_stack.pop()
        assert popped is self._sem_poison
        self.nc.clear_and_free_semaphores(list(self.sems.allocated().values()))
        self.nc.all_engine_barrier()


def _split_excess_waits(nc, maxw=1):
    """walrus in this container caps sync waits per instruction; move extras
    onto preceding same-engine NOPs (waits execute in program order)."""
    nid = 0
    for bb in nc.m.functions[0].blocks:
        insts = bb.instructions
        i = 0
        while i < len(insts):
            inst = insts[i]
            si = inst.sync_info
            nw = len(si.on_wait) if si is not None and si.on_wait else 0
            if nw > maxw:
                waits = list(si.on_wait)
                si.on_wait = waits[-maxw:]
                extra = waits[:-maxw]
                pos = i
                for k in range(0, len(extra), maxw):
                    nop = mybir.InstNoOp(
                        name=f"I-waitsplit-{nid}", ins=[], outs=[]
                    )
                    nop.engine = inst.engine
                    nop.sync_info = mybir.SyncInfo(
                        on_wait=extra[k : k + maxw], on_update=[]
                    )
                    insts.insert(pos, nop)
                    nc.register_instruction(nop)
                    pos += 1
                    i += 1
                    nid += 1
            i += 1


def build_program():
    nc = bass.Bass(trn_type="TRN2")

    xT = nc.dram_tensor("xT", [HID, S], F32, kind="ExternalInput")
    wfT = nc.dram_tensor("wfT", [HID, NFUSED], BF16, kind="ExternalInput")
    # wv / wa / wff pre-flattened on host to single-DMA layouts
    wvT = nc.dram_tensor("wvT", [128, 6 * VCOLS], BF16, kind="ExternalInput")
    waT = nc.dram_tensor("waT", [HD, HPC * HID], BF16, kind="ExternalInput")
    wffT = nc.dram_tensor("wffT", [128, 3 * HID], BF16, kind="ExternalInput")
    cosT = nc.dram_tensor("cosT", [128, S], BF16, kind="ExternalInput")
    sinT = nc.dram_tensor("sinT", [128, S], BF16, kind="ExternalInput")
    # rr [128,128] with rr64 packed at rows 0:64, cols 128:192
    rrT = nc.dram_tensor("rrT", [128, 192], BF16, kind="ExternalInput")
    # nw cols 0:6, rmask cols 6:12
    nwrm = nc.dram_tensor("nwrm", [128, 12], F32, kind="ExternalInput")
    # wq01 cols 0:2, wk01 cols 2:4
    wqk01 = nc.dram_tensor("wqk01", [128, 4], BF16, kind="ExternalInput")
    # wq2 col 0, wk2 col 1
    wqk2 = nc.dram_tensor("wqk2", [HD, 2], BF16, kind="ExternalInput")
    eb2 = nc.dram_tensor("eb2", [2, 128], BF16, kind="ExternalInput")
    outT = nc.dram_tensor("outT", [HID, S], F32, kind="ExternalOutput")

    with TileContextSplitDrain(nc) as tc, nc.allow_low_precision(
        reason="bf16 activations; accumulation and stats stay fp32"
    ):
        with (
            tc.tile_pool(name="big", bufs=1) as pbg,
            tc.tile_pool(name="wts", bufs=1) as pwt,
            tc.tile_pool(name="scr", bufs=3) as pscr,
            tc.tile_pool(name="probs", bufs=16) as ppr,
            tc.tile_pool(name="outp", bufs=4) as pout,
            tc.tile_pool(name="small", bufs=1) as psm,
        ):
            # ---- persistent bf16 tiles -----------------------------------
            xn = [pbg.tile([128, S], BF16, name=f"xn{c}", tag=f"xn{c}")
                  for c in range(6)]
            ffa = [pbg.tile([128, S], BF16, name=f"ffa{i}", tag=f"ffa{i}")
                   for i in range(3)]
            vx = [pbg.tile([128, VCOLS], BF16, name=f"vx{j}", tag=f"vx{j}")
                  for j in range(14)]
            qab = pbg.tile([128, S], BF16, name="qab", tag="qab")
            kab = pbg.tile([128, S], BF16, name="kab", tag="kab")
            q2t = pbg.tile([HD, S], BF16, name="q2t", tag="q2t")
            k2t = pbg.tile([HD, S], BF16, name="k2t", tag="k2t")
            cosb = pbg.tile([128, S], BF16, name="cosb", tag="cosb")
            sinb = pbg.tile([128, S], BF16, name="sinb", tag="sinb")
            tsin = pbg.tile([128, S], BF16, name="tsin", tag="tsin")
            tcos = pbg.tile([128, S], BF16, name="tcos", tag="tcos")
            sqA = pbg.tile([128, S], BF16, name="sqA", tag="sqA")
            sqB = pbg.tile([HD, S], BF16, name="sqB", tag="sqB")
            att3 = [pbg.tile([HD, S], BF16, name=f"att{h}", tag=f"att{h}")
                    for h in range(3)]
            rqs = pbg.tile([2, 2 * S], BF16, name="rqs", tag="rqs")
            e2 = pbg.tile([2, 128], BF16, name="e2", tag="e2")
            e1b = pbg.tile([65, HD], BF16, name="e1b", tag="e1b")
            dsb = pbg.tile([65, HALF], BF16, name="dsb", tag="dsb")
            sqsc = pbg.tile([128, S], BF16, name="sqsc", tag="sqsc")

            rrm = pwt.tile([128, 192], BF16, name="rrm", tag="rrm")
            rr = rrm[:, 0:128]
            rr64 = rrm[0:HD, 128:192]
            wf = [pwt.tile([128, NFUSED], BF16, name=f"wf{c}", tag=f"wf{c}")
                  for c in range(6)]
            wvm = pwt.tile([128, 6 * VCOLS], BF16, name="wvm", tag="wvm")
            wam = pwt.tile([HD, HPC * HID], BF16, name="wam", tag="wam")
            wffm = pwt.tile([128, 3 * HID], BF16, name="wffm", tag="wffm")
            wqk01t = pwt.tile([128, 4], BF16, name="wqk01t", tag="wqk01t")
            wq01t = wqk01t[:, 0:2]
            wk01t = wqk01t[:, 2:4]
            wqk2t = pwt.tile([HD, 2], BF16, name="wqk2t", tag="wqk2t")
            wq2t = wqk2t[:, 0:1]
            wk2t = wqk2t[:, 1:2]

            nwrmt = psm.tile([128, 12], F32, name="nwrmt", tag="nwrmt")
            nwt = nwrmt[:, 0:6]
            rmk = nwrmt[:, 6:12]
            ss6 = psm.tile([128, 6], F32, name="ss6", tag="ss6")
            scale6 = psm.tile([128, 6], F32, name="scale6", tag="scale6")
            risc0 = psm.tile([128, 6], F32, name="risc0", tag="risc0")
            risc = psm.tile([128, 6], F32, name="risc", tag="risc")
            rk01 = psm.tile([128, 28], F32, name="rk01", tag="rk01")
            rk2 = psm.tile([128, 14], F32, name="rk2", tag="rk2")
            lnq = psm.tile([2, 512], F32, name="lnq", tag="lnq")
            cgn = psm.tile([128, 1], F32, name="cgn", tag="cgn")
            cln2 = psm.tile([2, 1], F32, name="cln2", tag="cln2")
            cln1 = psm.tile([1, 1], F32, name="cln1", tag="cln1")
            cl64 = psm.tile([128, 1], F32, name="cl64", tag="cl64")

            nc.vector.memset(e1b[64:65, :], 1.0)
            nc.vector.memset(cgn[:], EPS_GN)
            nc.vector.memset(cln2[:], EPS_LN)
            nc.vector.memset(cln1[:], EPS_LN)
            nc.vector.memset(cl64[:], 64.0 * EPS_LN)

            # tiny tables on the scalar DGE queue (keeps the SP queue free
            # for the x/wf stream); nwrm first — needed by chunk-0 stats
            nc.scalar.dma_start(nwrmt[:], nwrm[:])
            nc.scalar.dma_start(wqk01t[:], wqk01[:])
            nc.scalar.dma_start(wqk2t[:], wqk2[:])
            nc.scalar.dma_start(e2[:], eb2[:])
            nc.scalar.dma_start(rrm[:], rrT[:])

            with tc.tile_pool(name="xr", bufs=1) as pxr:
                xraw = [pxr.tile([128, S], F32, name=f"xr{c}", tag=f"xr{c}")
                        for c in range(6)]

                # interleaved x/wf chunk stream: the double-buffered q01/k01
                # PSUM groups pre-accumulate c-chunks as they land
                for c in range(6):
                    nc.sync.dma_start(xraw[c][:], xT[128 * c : 128 * (c + 1), :])
                    nc.sync.dma_start(wf[c][:], wfT[128 * c : 128 * (c + 1), :])
                nc.sync.dma_start(cosb[:], cosT[:])
                nc.sync.dma_start(sinb[:], sinT[:])
                nc.sync.dma_start(wvm[:], wvT[:])
                nc.sync.dma_start(wam[:], waT[:])
                nc.sync.dma_start(wffm[:], wffT[:])

                # ---- per-chunk RMSGroupNorm stats + normalized x ---------
                for c in range(6):
                    nc.scalar.activation(
                        sqsc[:],
                        xraw[c][:],
                        AF.Square,
                        accum_out=ss6[:, c : c + 1],
                    )
                    # std = sqrt(ss/S + eps); scale6 = nw/std; risc = std/nw
                    nc.scalar.activation(
                        ss6[:, c : c + 1], ss6[:, c : c + 1], AF.Sqrt,
                        bias=cgn[:], scale=1.0 / S,
                    )
                    nc.vector.reciprocal(
                        risc0[:, c : c + 1], ss6[:, c : c + 1]
                    )
                    nc.vector.tensor_mul(
                        scale6[:, c : c + 1], risc0[:, c : c + 1],
                        nwt[:, c : c + 1],
                    )
                    nc.vector.reciprocal(
                        risc0[:, c : c + 1], scale6[:, c : c + 1]
                    )
                    nc.vector.tensor_mul(
                        risc[:, c : c + 1], risc0[:, c : c + 1], rmk[:, c : c + 1]
                    )
                    nc.vector.tensor_scalar(
                        xn[c][:], xraw[c][:], scale6[:, c : c + 1], None, ALU.mult
                    )

            # ---- fused projection: q01, k01, q2, k2 ----------------------
            qk_dst = [qab, kab, q2t, k2t]
            with tc.tile_pool(name="psA2", bufs=6, space="PSUM") as psA2:
                for o in range(4):
                    for soff, slen in S_BLOCKS:
                        pt = psA2.tile([128, 512], F32, name="mm", tag="mm")
                        acc = pt[:, :slen]
                        for c in range(6):
                            nc.tensor.matmul(
                                acc,
                                wf[c][:, 128 * o : 128 * (o + 1)],
                                xn[c][:, soff : soff + slen],
                                start=(c == 0),
                                stop=(c == 5),
                            )
                        if o < 2:
                            if o == 0:
                                nc.vector.tensor_copy(
                                    qk_dst[o][:, soff : soff + slen], acc
                                )
                            else:
                                nc.scalar.activation(
                                    qk_dst[o][:, soff : soff + slen], acc,
                                    AF.Copy,
                                )
                        else:
                            nc.scalar.activation(
                                qk_dst[o][:, soff : soff + slen], acc[0:HD, :],
                                AF.Copy,
                            )

            with tc.tile_pool(name="psA1", bufs=1, space="PSUM") as psA1:
                # ---- v projection (token-major) --------------------------
                for j, (toff, tlen) in enumerate(T_TILES):
                    pt = psA1.tile([128, 512], F32, name="mmv", tag="mm")
                    acc = pt[:tlen, :VCOLS]
                    for c in range(6):
                        nc.tensor.matmul(
                            acc,
                            xn[c][:, toff : toff + tlen],
                            wv[c][:],
                            start=(c == 0),
                            stop=(c == 5),
                        )
                    nc.scalar.activation(vx[j][:tlen, :], acc, AF.Copy)
                    # ones columns for the softmax denominators
                    nc.vector.memset(vx[j][:tlen, HD : VCOLS : HD + 1], 1.0)

                # ---- q/k layernorm rstd stats ----------------------------
                with tc.tile_pool(name="psStat", bufs=1, space="PSUM") as psV:
                    # q side: rstd rows [2, S] (h0/h1) + [1, S] (h2);
                    # rsqrt as exp(-0.5 ln(var + eps))
                    nc.vector.tensor_mul(sqA[:], qab[:], qab[:])
                    for soff, slen in S_BLOCKS:
                        pt = psV.tile([2, 512], F32, name="vq", tag="vq")
                        nc.tensor.matmul(
                            pt[:, :slen], wq01t[:], sqA[:, soff : soff + slen]
                        )
                        nc.scalar.activation(
                            lnq[0:2, :slen], pt[:, :slen], AF.Ln, bias=cln2[:]
                        )
                        nc.scalar.activation(
                            rqs[0:2, soff : soff + slen], lnq[0:2, :slen],
                            AF.Exp, scale=-0.5,
                        )
                    nc.vector.tensor_mul(sqB[:], q2t[:], q2t[:])
                    for soff, slen in S_BLOCKS:
                        pt = psV.tile([2, 512], F32, name="vq2", tag="vq")
                        nc.tensor.matmul(
                            pt[0:1, :slen], wq2t[:], sqB[:, soff : soff + slen]
                        )
                        nc.scalar.activation(
                            lnq[0:1, :slen], pt[0:1, :slen], AF.Ln, bias=cln1[:]
                        )
                        nc.scalar.activation(
                            rqs[0:1, S + soff : S + soff + slen],
                            lnq[0:1, :slen],
                            AF.Exp, scale=-0.5,
                        )

                    # k side: rstd/8 columns, consolidated Rsqrts
                    nc.vector.tensor_mul(sqA[:], kab[:], kab[:])
                    nc.vector.tensor_mul(sqB[:], k2t[:], k2t[:])
                    vst = psV.tile([128, 48], F32, name="vst", tag="vst")
                    nc.vector.memset(vst[:], 1.0)
                    for j, (toff, tlen) in enumerate(T_TILES):
                        nc.tensor.matmul(
                            vst[:tlen, 2 * j : 2 * j + 2],
                            sqA[:, toff : toff + tlen],
                            wk01t[:],
                        )
                        nc.tensor.matmul(
                            vst[:tlen, 28 + j : 29 + j],
                            sqB[:, toff : toff + tlen],
                            wk2t[:],
                        )
                    nc.scalar.activation(
                        rk01[:], vst[:, 0:28], AF.Ln, bias=cl64[:]
                    )
                    nc.scalar.activation(
                        rk01[:], rk01[:], AF.Exp, scale=-0.5
                    )
                    nc.scalar.activation(
                        rk2[:], vst[:, 28:42], AF.Ln, bias=cl64[:]
                    )
                    nc.scalar.activation(
                        rk2[:], rk2[:], AF.Exp, scale=-0.5
                    )

                # rstd_q broadcasts on the idle GPSIMD engine
                nc.gpsimd.partition_broadcast(qrs_b[0:HD, :], rqs[0:1, 0:S])
                nc.gpsimd.partition_broadcast(qrs_b[HD:128, :], rqs[1:2, 0:S])
                nc.gpsimd.partition_broadcast(q2rs_b[:], rqs[0:1, S : 2 * S])

                with tc.tile_pool(name="psB", bufs=1, space="PSUM") as psB:

                    def rope(dst, nrow, rmat, rs_b):
                        for soff, slen in S_BLOCKS:
                            pt = psB.tile([128, 512], F32, name="rot", tag="rot")
                            nc.tensor.matmul(
                                pt[:nrow, :slen],
                                rmat[:],
                                dst[:, soff : soff + slen],
                            )
                            nc.vector.tensor_mul(
                                tsin[:nrow, soff : soff + slen],
                                pt[:nrow, :slen],
                                sinb[:nrow, soff : soff + slen],
                            )
                        nc.vector.tensor_mul(
                            tcos[:nrow, :], dst[:], cosb[:nrow, :]
                        )
                        if rs_b is None:
                            nc.vector.tensor_add(
                                dst[:], tsin[:nrow, :], tcos[:nrow, :]
                            )
                        else:
                            nc.vector.tensor_add(
                                tsin[:nrow, :], tsin[:nrow, :], tcos[:nrow, :]
                            )
                            nc.vector.tensor_mul(
                                dst[:], tsin[:nrow, :], rs_b[:]
                            )

                    rope(qab, 128, rr, qrs_b)
                    rope(kab, 128, rr, None)
                    rope(q2t, HD, rr64, q2rs_b)
                    rope(k2t, HD, rr64, None)

                    # ---- attention ---------------------------------------
                    qsl = [qab[0:HD], qab[HD:128], q2t[:]]
                    ksl = [kab[0:HD], kab[HD:128], k2t[:]]

                    with (
                        tc.tile_pool(name="psS", bufs=2, space="PSUM") as psS,
                        tc.tile_pool(name="psAV", bufs=1, space="PSUM") as psAV,
                    ):
                        for hf in range(2):
                            for h in range(HPC):
                                hoff = HALF * hf
                                av = psAV.tile(
                                    [65, HALF], F32, name="av", tag="av"
                                )
                                for j, (toff, tlen) in enumerate(T_TILES):
                                    pb = ppr.tile(
                                        [128, HALF], BF16, name="pb", tag="pb"
                                    )
                                    sc = psS.tile(
                                        [128, HALF], F32, name="sc", tag="sc"
                                    )
                                    for aoff, alen in HSUBS:
                                        nc.tensor.matmul(
                                            sc[:tlen, aoff : aoff + alen],
                                            ksl[h][:, toff : toff + tlen],
                                            qsl[h][
                                                :, hoff + aoff : hoff + aoff + alen
                                            ],
                                        )
                                    scl = (
                                        rk01[:tlen, 2 * j + h : 2 * j + h + 1]
                                        if h < 2
                                        else rk2[:tlen, j : j + 1]
                                    )
                                    nc.scalar.activation(
                                        pb[:tlen, :], sc[:tlen, :], AF.Exp,
                                        scale=scl,
                                    )
                                    for aoff, alen in HSUBS:
                                        nc.tensor.matmul(
                                            av[:, aoff : aoff + alen],
                                            vx[j][
                                                :tlen,
                                                (HD + 1) * h : (HD + 1) * (h + 1),
                                            ],
                                            pb[:tlen, aoff : aoff + alen],
                                            start=(j == 0),
                                            stop=(j == 13),
                                        )
                                # normalize: recip of denominator row, GPSIMD
                                # broadcast, single fused evac multiply
                                nc.vector.reciprocal(
                                    dsb[64:65, :], av[64:65, :]
                                )
                                for aoff, alen in HSUBS:
                                    nc.vector.tensor_copy(
                                        att3[h][
                                            :, hoff + aoff : hoff + aoff + alen
                                        ],
                                        av[0:HD, aoff : aoff + alen],
                                    )
                                for aoff, alen in HSUBS:
                                    po = psB.tile(
                                        [128, 512], F32, name="pod", tag="rb"
                                    )
                                    nc.tensor.matmul(
                                        po[0:HD, :alen],
                                        e1b[64:65, :],
                                        dsb[64:65, aoff : aoff + alen],
                                    )
                                    nc.vector.tensor_mul(
                                        att3[h][
                                            :, hoff + aoff : hoff + aoff + alen
                                        ],
                                        att3[h][
                                            :, hoff + aoff : hoff + aoff + alen
                                        ],
                                        po[0:HD, :alen],
                                    )

                    # ---- ffx/gate fused blocks + output blocks, both on
                    # the psA1 slot (PE filler during the ACT-bound attention
                    # window; output halves start as soon as the matching
                    # attention halves and ff columns are done) -------------
                    def filler_block(soff, slen):
                        for o in range(4, 10):
                            pt = psA1.tile([128, 512], F32, name="mmf", tag="mm")
                            acc = pt[:, :slen]
                            for c in range(6):
                                nc.tensor.matmul(
                                    acc,
                                    wf[c][:, 128 * o : 128 * (o + 1)],
                                    xn[c][:, soff : soff + slen],
                                    start=(c == 0),
                                    stop=(c == 5),
                                )
                            if o < 7:
                                nc.vector.tensor_copy(
                                    ffa[o - 4][:, soff : soff + slen], acc
                                )
                            else:
                                gs = pscr.tile(
                                    [128, 512], BF16, name="gs", tag="gs"
                                )
                                nc.scalar.activation(
                                    gs[:, :slen], acc, AF.Silu
                                )
                                nc.vector.tensor_mul(
                                    ffa[o - 7][:, soff : soff + slen],
                                    ffa[o - 7][:, soff : soff + slen],
                                    gs[:, :slen],
                                )

                    def out_block(soff, slen, pools):
                        for o in range(6):
                            pool = pools[o % len(pools)]
                            pt = pool.tile([128, 512], F32, name="oc", tag="oc")
                            acc = pt[:, :slen]
                            for h in range(3):
                                nc.tensor.matmul(
                                    acc,
                                    wam[
                                        :,
                                        HID * h + 128 * o : HID * h + 128 * (o + 1),
                                    ],
                                    att3[h][:, soff : soff + slen],
                                    start=(h == 0),
                                    stop=False,
                                )
                            for c in range(3):
                                nc.tensor.matmul(
                                    acc,
                                    wffm[
                                        :,
                                        HID * c + 128 * o : HID * c + 128 * (o + 1),
                                    ],
                                    ffa[c][:, soff : soff + slen],
                                    start=False,
                                    stop=(c == 2),
                                )
                            ob = pout.tile(
                                [128, 512], F32, name="obt", tag="obt"
                            )
                            nc.vector.scalar_tensor_tensor(
                                ob[:, :slen],
                                xn[o][:, soff : soff + slen],
                                risc[:, o : o + 1],
                                acc,
                                ALU.mult,
                                ALU.add,
                            )
                            nc.sync.dma_start(
                                outT[128 * o : 128 * (o + 1), soff : soff + slen],
                                ob[:, :slen],
                            )

                    filler_block(*S_BLOCKS[0])
                    filler_block(*S_BLOCKS[1])
                    filler_block(*S_BLOCKS[2])
                    filler_block(*S_BLOCKS[3])
                    psB.release()
                    psC = tc.alloc_tile_pool(name="psC", bufs=4, space="PSUM")
                    for soff, slen in HALVES[0] + HALVES[1]:
                        out_block(soff, slen, [psC])
                    psC.release()
                    psA1.release()
    _split_excess_waits(nc)
    return nc


# ---------------------------------------------------------------------------
# host-side preparation
# ---------------------------------------------------------------------------


def _axial_freqs():
    base = np.linspace(1.0, MAX_FREQ / 2, 8) * math.pi

    def ax(n):
        pos = np.linspace(-1.0, 1.0, n)
        return np.repeat(pos[:, None] * base[None, :], 2, axis=-1)

    fH = np.broadcast_to(ax(H)[:, None, None, :], (H, W, D, 16))
    fW = np.broadcast_to(ax(W)[None, :, None, :], (H, W, D, 16))
    fD = np.broadcast_to(ax(D)[None, None, :, :], (H, W, D, 16))
    return np.concatenate((fH, fW, fD), axis=-1).reshape(S, ROT)


def _bf16(a):
    import ml_dtypes

    return np.ascontiguousarray(np.asarray(a, np.float32)).astype(
        ml_dtypes.bfloat16
    )


def _prep_core_inputs(x, norm1_w, w_fused, b_fused, q_gamma, q_beta, k_gamma,
                      k_beta, w_attn, w_ff, b_ff):
    """Returns list of 8 in_maps (core = b*4 + r)."""
    f64 = np.float64
    w_fused = np.asarray(w_fused, f64)
    q_gamma = np.asarray(q_gamma, f64)
    k_gamma = np.asarray(k_gamma, f64)

    if np.any(np.asarray(b_fused)) or np.any(np.asarray(b_ff)):
        raise NotImplementedError("nonzero biases not supported by this kernel")
    if np.any(np.asarray(q_beta)) or np.any(np.asarray(k_beta)):
        raise NotImplementedError("nonzero q/k beta not supported by this kernel")
    if np.any(q_gamma == 0) or np.any(k_gamma == 0):
        raise NotImplementedError("zero gamma not supported by this kernel")

    M = np.eye(HD) - np.ones((HD, HD)) / HD
    Aq = np.diag(q_gamma) @ M
    Ak = np.diag(k_gamma) @ M
    R = np.zeros((HD, HD))
    for i in range(ROT // 2):
        R[2 * i, 2 * i + 1] = -1.0
        R[2 * i + 1, 2 * i] = 1.0
    R2 = np.zeros((128, 128))
    R2[0:64, 0:64] = R
    R2[64:128, 64:128] = R

    freqs = _axial_freqs()
    cos64 = np.ones((HD, S))
    sin64 = np.zeros((HD, S))
    cos64[:ROT, :] = np.cos(freqs).T
    sin64[:ROT, :] = np.sin(freqs).T
    cosT = _bf16(np.vstack([cos64, cos64]))
    sinT = _bf16(np.vstack([sin64, sin64]))

    wq_full = w_fused[MLP : MLP + HID]
    wk_full = w_fused[MLP + HID : MLP + 2 * HID]
    wv_full = w_fused[MLP + 2 * HID :]
    ffx_full = w_fused[: MLP // 2]
    gate_full = w_fused[MLP // 2 : MLP]

    nw = np.asarray(norm1_w, np.float32).reshape(6, 128).T
    wqk01 = np.zeros((128, 4))
    wqk01[0:64, 0] = 1.0 / (HD * q_gamma**2)
    wqk01[64:128, 1] = 1.0 / (HD * q_gamma**2)
    wqk01[0:64, 2] = 1.0 / k_gamma**2
    wqk01[64:128, 3] = 1.0 / k_gamma**2
    wqk2 = np.zeros((HD, 2))
    wqk2[:, 0] = 1.0 / (HD * q_gamma**2)
    wqk2[:, 1] = 1.0 / k_gamma**2
    rrm = np.zeros((128, 192))
    rrm[:, 0:128] = R2.T
    rrm[0:64, 128:192] = R.T
    eb2_np = np.zeros((2, 128))
    eb2_np[0, 0:64] = 1.0
    eb2_np[1, 64:128] = 1.0

    pad = np.zeros((64, HID))
    in_maps = []
    for core in range(N_CORES):
        b, r = divmod(core, TP)
        hs = [HPC * r + i for i in range(HPC)]
        q3 = [Aq @ wq_full[HD * h : HD * (h + 1)] for h in hs]
        k3 = [Ak @ wk_full[HD * h : HD * (h + 1)] for h in hs]
        ffx = ffx_full[FFPC * r : FFPC * (r + 1)]
        gate = gate_full[FFPC * r : FFPC * (r + 1)]
        wfT_np = np.vstack(
            [q3[0], q3[1], k3[0], k3[1], q3[2], pad, k3[2], pad, ffx, gate]
        ).T
        wv_mat = np.zeros((VCOLS, HID))
        for i, h in enumerate(hs):
            wv_mat[(HD + 1) * i : (HD + 1) * i + HD] = wv_full[HD * h : HD * (h + 1)]
        # [768, VCOLS] -> [128, 6*VCOLS] (chunk-c columns side by side)
        wvT_np = (
            wv_mat.T.reshape(6, 128, VCOLS)
            .transpose(1, 0, 2)
            .reshape(128, 6 * VCOLS)
        )
        acols = np.concatenate([np.arange(HD * h, HD * (h + 1)) for h in hs])
        waT_np = (
            np.asarray(w_attn, f64)[:, acols]
            .T.reshape(3, HD, HID)
            .transpose(1, 0, 2)
            .reshape(HD, 3 * HID)
        )
        wffT_np = (
            np.asarray(w_ff, f64)[:, FFPC * r : FFPC * (r + 1)]
            .T.reshape(3, 128, HID)
            .transpose(1, 0, 2)
            .reshape(128, 3 * HID)
        )
        nwrm = np.zeros((128, 12), np.float32)
        nwrm[:, 0:6] = nw
        nwrm[:, 6:12] = 1.0 if r == 0 else 0.0
        in_maps.append(
            {
                "xT": np.ascontiguousarray(
                    np.asarray(x[b], np.float32).reshape(HID, S)
                ),
                "wfT": _bf16(wfT_np),
                "wvT": _bf16(wvT_np),
                "waT": _bf16(waT_np),
                "wffT": _bf16(wffT_np),
                "cosT": cosT,
                "sinT": sinT,
                "rrT": _bf16(rrm),
                "nwrm": nwrm,
                "wqk01": _bf16(wqk01),
                "wqk2": _bf16(wqk2),
                "eb2": _bf16(eb2_np),
            }
        )
    return in_maps


_NC_CACHE = {}


def get_program():
    if "nc" not in _NC_CACHE:
        _NC_CACHE["nc"] = build_program()
    return _NC_CACHE["nc"]


def kernel(**inputs) -> np.ndarray:
    nc = get_program()
    in_maps = _prep_core_inputs(**inputs)
    res = bass_utils.run_bass_kernel_spmd(nc, in_maps, core_ids=list(range(N_CORES)))
    out = np.zeros((B, HID, H, W, D), np.float32)
    for core in range(N_CORES):
        b = core // TP
        out[b] += res.results[core]["outT"].reshape(HID, H, W, D)
    return out
